# revision 1
# baseline (speedup 1.0000x reference)
"""Chamfer distance loss kernel for 8 Trainium2 NeuronCores.

Problem: points1 [8, 4096, 3], points2 [8, 4096, 3] (f32).
  dist[b,n,m] = ||p1[b,n]||^2 + ||p2[b,m]||^2 - 2 p1.p2
  loss = (mean_n,b(min_m dist) + mean_m,b(min_n dist)) / 8     (scalar f32)

Sharding: data-parallel over batch B: core b handles batch b.

Per-core algorithm (flash-style, nothing materialized in HBM):
  Host lifts each point cloud to K=8 rows so that the *negated* distance
  matrix is one K=8 matmul:  -d[n,m] = sum_k la[k,n] * lb[k,m]
  Device loop over 32 row-strips of 128 points1 (groups of 4):
     PE:  8 matmuls (N=512, fp32, 4-way packed via tile_position)
          -> PSUM strip [128, 4096] f32 (2 halves)
     ACT: cast PSUM f32 -> SBUF fp16 strip
     DVE: colacc = max(colacc, strip) elementwise (fp16 2x mode)
          rowmax via fold-max tree 4096->128, 3D-AP ops spanning the group
  Tail: colacc partition-max via 32 PE transposes into one PSUM f16 tile,
        4 quarter tensor_reduce(max), fused sum, gpsimd partition reduce,
        one f32 scalar ( -(rowsum+colsum) ) DMA'd out.
Host: loss = -sum(partials) / (B*B*N).

`parts` builds ablated variants for engine-time attribution (timing only):
  "mm" PE only | "mm+act" +casts | "+col" +colacc | "+row" +folds | "full".
"""

import sys
import numpy as np

for _p in ("/opt/trn_rl_repo", "/root/.axon_site/_ro/trn_rl_repo"):
    if _p not in sys.path:
        sys.path.insert(0, _p)

B = 8
N = 4096
D = 3
K = 8
P = 128
NSTRIP = N // P          # 32
MM_FREE = 512            # fp32 matmul moving-operand max
MHALF = 2048             # half strip (4 PSUM banks)

_NC_CACHE = {}


def _build_nc(repeat=1, parts="full", tail="new", group=4, rowred="fold",
              colacc_half=True, cast_parts=1, fold_scope="group",
              colskip=0, foldskip=0, sbufs=None, scrbufs=2, dvecast=0,
              tailred="gpsimd", colacc2=0, foldstop=128, fold_sub=2):
    import contextlib

    import concourse.bacc as bacc
    import concourse.tile as tile
    from concourse import bass_isa, mybir

    F16 = mybir.dt.float16
    F32 = mybir.dt.float32
    MAX = mybir.AluOpType.max
    ADD = mybir.AluOpType.add

    do_act = parts != "mm"
    do_col = parts in ("mm+act+col", "full")
    do_row = parts in ("mm+act+row", "full")
    do_tail = parts == "full"

    nc = bacc.Bacc(
        "TRN2", target_bir_lowering=False, debug=False, num_devices=B
    )
    la = nc.declare_dram_parameter("la", [K, N], F32, isOutput=False)
    lb = nc.declare_dram_parameter("lb", [K, N], F32, isOutput=False)
    ident = nc.declare_dram_parameter("ident", [P, P], F16, isOutput=False)
    out_shape = [P, 1] if tailred == "host" else [1, 1]
    out = nc.declare_dram_parameter("partial", out_shape, F32, isOutput=True)

    with tile.TileContext(nc) as tc:
        with (
            tc.tile_pool(name="consts", bufs=1) as consts,
            tc.tile_pool(
                name="strips",
                bufs=sbufs if sbufs else (3 if group <= 2 else 2),
            ) as strips,
            tc.tile_pool(name="scr", bufs=scrbufs) as scr,
            tc.tile_pool(name="accs", bufs=1) as accs,
            tc.tile_pool(name="psum", bufs=2, space="PSUM") as psum,
        ):
            la_sb = consts.tile([3 * 32 + K, N], F32)
            lb_sb = consts.tile([3 * 32 + K, N], F32)
            for q in range(4):
                nc.sync.dma_start(out=la_sb[32 * q : 32 * q + K, :], in_=la[:])
                nc.scalar.dma_start(out=lb_sb[32 * q : 32 * q + K, :], in_=lb[:])
            idt = consts.tile([P, P], F16)
            nc.gpsimd.dma_start(out=idt[:], in_=ident[:])

            loop_ctx = (
                tc.For_i(0, repeat, 1) if repeat != 1 else contextlib.nullcontext()
            )
            with loop_ctx:
                colacc = accs.tile([P, N], F16)
                if colacc2:
                    colaccB = accs.tile([P, N], F16)
                else:
                    colaccB = None
                rowacc = accs.tile([P, NSTRIP * foldstop], F16)
                if foldskip:
                    nc.vector.memset(rowacc[:], 0.0)
                summ = accs.tile([P, 2 * NSTRIP], F32)
                colred = accs.tile([P, NSTRIP], F32)
                if rowred == "tmr":
                    mask_n = accs.tile([P, 1], F32)
                    nc.vector.memset(mask_n[:], float(N))

                def emit_mms(i, h, ph):
                    for j in range(MHALF // MM_FREE):
                        m0 = j * MM_FREE
                        nc.tensor.matmul(
                            ph[:, m0 : m0 + MM_FREE],
                            lhsT=la_sb[32 * j : 32 * j + K, i * P : (i + 1) * P],
                            rhs=lb_sb[
                                32 * j : 32 * j + K,
                                h * MHALF + m0 : h * MHALF + m0 + MM_FREE,
                            ],
                            start=True,
                            stop=True,
                            tile_position=(32 * j, 0),
                        )

                G = group
                for ip in range(NSTRIP // G):
                    dstrip = strips.tile([P, G, N], F16, tag="strip")
                    last_sub = ip == NSTRIP // G - 1
                    for s in range(G):
                        i = G * ip + s
                        for h in range(2):
                            ph = psum.tile([P, MHALF], F32, tag="ph")
                            emit_mms(i, h, ph)
                            if do_act:
                                dv = dvecast if h == 1 else 0
                                aw = MHALF - dv
                                cw = aw // cast_parts
                                for cp in range(cast_parts):
                                    nc.scalar.copy(
                                        dstrip[
                                            :, s,
                                            h * MHALF + cp * cw
                                            : h * MHALF + (cp + 1) * cw,
                                        ],
                                        ph[:, cp * cw : (cp + 1) * cw],
                                    )
                                if dv:
                                    nc.vector.tensor_copy(
                                        dstrip[:, s, h * MHALF + aw : (h + 1) * MHALF],
                                        ph[:, aw:MHALF],
                                    )
                            if do_col and colacc_half and not (colskip and s % 2):
                                cacc = (
                                    colaccB
                                    if (colacc2 and i % 2 == 1)
                                    else colacc
                                )
                                init = (ip == 0 and s == 0) or (
                                    colacc2 and ip == 0 and s == 1
                                )
                                nq = 2 if colacc_half == 2 else 1
                                for qq in range(nq):
                                    hs = h * MHALF + qq * (MHALF // nq)
                                    he = hs + MHALF // nq
                                    if init:
                                        nc.vector.tensor_copy(
                                            cacc[:, hs:he], dstrip[:, s, hs:he]
                                        )
                                    elif last_sub and s == G - 1 and nq == 1:
                                        for q in range(2):
                                            qs = hs + q * (N // 4)
                                            qe = qs + N // 4
                                            nc.vector.tensor_tensor(
                                                cacc[:, qs:qe],
                                                cacc[:, qs:qe],
                                                dstrip[:, s, qs:qe],
                                                op=MAX,
                                            )
                                    else:
                                        nc.vector.tensor_tensor(
                                            cacc[:, hs:he],
                                            cacc[:, hs:he],
                                            dstrip[:, s, hs:he],
                                            op=MAX,
                                        )
                        if do_col and not colacc_half:
                            if ip == 0 and s == 0:
                                nc.vector.tensor_copy(colacc[:], dstrip[:, s, :])
                            elif last_sub and s == G - 1:
                                for q in range(4):
                                    qs = q * (N // 4)
                                    qe = qs + N // 4
                                    nc.vector.tensor_tensor(
                                        colacc[:, qs:qe],
                                        colacc[:, qs:qe],
                                        dstrip[:, s, qs:qe],
                                        op=MAX,
                                    )
                            else:
                                nc.vector.tensor_tensor(
                                    colacc[:], colacc[:], dstrip[:, s, :], op=MAX
                                )
                        if do_row and rowred == "ts":
                            nc.vector.tensor_scalar(
                                out=dstrip[:, s, :],
                                in0=dstrip[:, s, :],
                                scalar1=-1.0e30,
                                scalar2=None,
                                op0=MAX,
                                op1=MAX,
                                accum_out=summ[:, i : i + 1],
                            )
                        elif do_row and rowred == "tmr":
                            mscr = scr.tile([P, N], F16, tag="mscr")
                            nc.vector.tensor_mask_reduce(
                                out=mscr[:],
                                in_=dstrip[:, s, :],
                                mask_start=0.0,
                                mask_end=mask_n[:],
                                scale=1.0,
                                accum_in=-1.0e30,
                                op=MAX,
                                accum_out=summ[:, i : i + 1],
                            )
                        if do_row and rowred == "fold" and fold_scope == "strip":
                            w = N // 2
                            fsrc = dstrip[:, s, :]
                            while w > 128:
                                fdst = scr.tile([P, w], F16, tag=f"sfold{w}")
                                nc.vector.tensor_tensor(
                                    fdst[:], fsrc[:, 0:w],
                                    fsrc[:, w : 2 * w], op=MAX,
                                )
                                fsrc = fdst[:]
                                w //= 2
                            nc.vector.tensor_tensor(
                                rowacc[:, i * 128 : (i + 1) * 128],
                                fsrc[:, 0:128],
                                fsrc[:, 128:256],
                                op=MAX,
                            )
                        if (
                            do_row and rowred == "fold" and fold_sub == 2
                            and s % 2 == 1 and not foldskip
                        ):
                            sub0 = s - 1
                            w = N // 2
                            fsrc = dstrip[:, sub0 : sub0 + 2, :]
                            while w > foldstop:
                                fdst = scr.tile([P, 2, w], F16, tag=f"fold{w}")
                                nc.vector.tensor_tensor(
                                    fdst[:], fsrc[:, :, 0:w],
                                    fsrc[:, :, w : 2 * w], op=MAX,
                                )
                                fsrc = fdst[:]
                                w //= 2
                            i0 = G * ip + sub0
                            nc.vector.tensor_tensor(
                                rowacc[
                                    :, i0 * foldstop : (i0 + 2) * foldstop
                                ].rearrange("p (s w) -> p s w", s=2),
                                fsrc[:, :, 0:foldstop],
                                fsrc[:, :, foldstop : 2 * foldstop],
                                op=MAX,
                            )
                    if do_row and rowred == "fold" and fold_scope == "group" and fold_sub != 2 and not foldskip:
                        w = N // 2
                        src = dstrip
                        while w > foldstop:
                            dst = scr.tile([P, G, w], F16, tag=f"fold{w}")
                            nc.vector.tensor_tensor(
                                dst[:], src[:, :, 0:w], src[:, :, w : 2 * w],
                                op=MAX,
                            )
                            src = dst
                            w //= 2
                        nc.vector.tensor_tensor(
                            rowacc[
                                :, G * ip * foldstop : (G * ip + G) * foldstop
                            ].rearrange("p (s w) -> p s w", s=G),
                            src[:, :, 0:foldstop],
                            src[:, :, foldstop : 2 * foldstop],
                            op=MAX,
                        )

                # ---- tail ----
                if do_tail:
                    if foldskip:
                        nc.vector.memset(summ[:, 0:NSTRIP], 0.0)
                    elif rowred == "fold":
                        # per-strip rowmax: fold the 128 candidates per strip
                        # to 2 at 2x mode, then one small 1x reduce
                        rw = foldstop // 2
                        v = rowacc[:].rearrange("p (i w) -> p i w", w=foldstop)
                        while rw >= 2:
                            rdst = scr.tile([P, NSTRIP, rw], F16, tag=f"rfold{rw}")
                            nc.vector.tensor_tensor(
                                rdst[:], v[:, :, 0:rw], v[:, :, rw : 2 * rw],
                                op=MAX,
                            )
                            v = rdst[:]
                            rw //= 2
                        nc.vector.tensor_reduce(
                            out=summ[:, 0:NSTRIP],
                            in_=v,
                            axis=mybir.AxisListType.X,
                            op=MAX,
                        )
                    if tail == "new":
                        tailp = psum.tile([P, NSTRIP, P], F16, tag="ph")
                        for q in range(4):
                            if colacc2:
                                qs = q * (N // 4)
                                qe = qs + N // 4
                                nc.vector.tensor_tensor(
                                    colacc[:, qs:qe],
                                    colacc[:, qs:qe],
                                    colaccB[:, qs:qe],
                                    op=MAX,
                                )
                            for t in range(8):
                                k = 8 * q + t
                                nc.tensor.transpose(
                                    tailp[:, k, :],
                                    colacc[:, k * P : (k + 1) * P],
                                    idt[:],
                                )
                            nc.vector.tensor_reduce(
                                out=colred[:, 8 * q : 8 * q + 8],
                                in_=tailp[:, 8 * q : 8 * q + 8, :],
                                axis=mybir.AxisListType.X,
                                op=MAX,
                            )
                        nc.vector.tensor_copy(summ[:, NSTRIP : 2 * NSTRIP], colred[:])
                    else:
                        for k in range(NSTRIP):
                            tp = psum.tile([P, P], F16, tag="ph")
                            nc.tensor.transpose(
                                tp[:], colacc[:, k * P : (k + 1) * P], idt[:]
                            )
                            nc.vector.tensor_reduce(
                                out=summ[:, NSTRIP + k : NSTRIP + k + 1],
                                in_=tp[:],
                                axis=mybir.AxisListType.X,
                                op=MAX,
                            )
                    tot = accs.tile([P, 1], F32)
                    nc.vector.tensor_reduce(
                        out=tot[:], in_=summ[:], axis=mybir.AxisListType.X, op=ADD
                    )
                    if tailred == "host":
                        nc.sync.dma_start(out=out[:], in_=tot[:])
                    else:
                        tot_red = accs.tile([P, 1], F32)
                        nc.gpsimd.partition_all_reduce(
                            tot_red[:], tot[:], P, bass_isa.ReduceOp.add
                        )
                        nc.sync.dma_start(out=out[:], in_=tot_red[0:1, :])
                else:
                    # ablation: still produce the output tensor
                    tot_red = accs.tile([P, 1], F32)
                    nc.vector.memset(tot_red[:], 0.0)
                    nc.sync.dma_start(
                        out=out[:],
                        in_=tot_red[:] if tailred == "host" else tot_red[0:1, :],
                    )

    nc.compile()
    return nc


def get_nc(repeat=1, parts="full", tail="new", group=4, rowred="fold",
           colacc_half=True, cast_parts=1, fold_scope="group", colskip=0,
           foldskip=0, sbufs=None, scrbufs=2, dvecast=0, tailred="gpsimd",
           colacc2=0, foldstop=128, fold_sub=2, **_ignored):
    key = (repeat, parts, tail, group, rowred, colacc_half, cast_parts,
           fold_scope, colskip, foldskip, sbufs, scrbufs, dvecast, tailred,
           colacc2, foldstop, fold_sub)
    if key not in _NC_CACHE:
        _NC_CACHE[key] = _build_nc(
            repeat=repeat, parts=parts, tail=tail, group=group,
            rowred=rowred, colacc_half=colacc_half, cast_parts=cast_parts,
            fold_scope=fold_scope, colskip=colskip, foldskip=foldskip,
            sbufs=sbufs, scrbufs=scrbufs, dvecast=dvecast, tailred=tailred,
            colacc2=colacc2, foldstop=foldstop, fold_sub=fold_sub,
        )
    return _NC_CACHE[key]


def _lift(points1, points2):
    """Host-side O(N) prep: lifted vectors so -dist = la^T @ lb."""
    p1 = np.asarray(points1, dtype=np.float32)
    p2 = np.asarray(points2, dtype=np.float32)
    sq1 = np.sum(p1 * p1, axis=-1)  # [B, N]
    sq2 = np.sum(p2 * p2, axis=-1)  # [B, N]
    la = np.zeros((B, K, N), dtype=np.float32)
    lb = np.zeros((B, K, N), dtype=np.float32)
    la[:, 0, :] = sq1
    la[:, 1, :] = 1.0
    la[:, 2:5, :] = np.transpose(p1, (0, 2, 1))
    lb[:, 0, :] = -1.0
    lb[:, 1, :] = -sq2
    lb[:, 2:5, :] = 2.0 * np.transpose(p2, (0, 2, 1))
    return la, lb


def _in_maps(points1, points2):
    la, lb = _lift(points1, points2)
    ident = np.eye(P, dtype=np.float16)
    return [
        {
            "la": np.ascontiguousarray(la[b]),
            "lb": np.ascontiguousarray(lb[b]),
            "ident": ident,
        }
        for b in range(B)
    ]


def kernel(points1, points2):
    from concourse.bass_utils import run_bass_kernel_spmd

    in_maps = _in_maps(points1, points2)
    nc = get_nc()
    res = run_bass_kernel_spmd(nc, in_maps, list(range(B))).results
    tot = -sum(float(np.sum(res[b]["partial"])) for b in range(B))
    loss = tot / (B * B * N)
    return np.float32(loss)



# revision 2
# speedup vs baseline: 2.3552x; 2.3552x over previous
"""Chamfer distance loss kernel for 8 Trainium2 NeuronCores.

Problem: points1 [8, 4096, 3], points2 [8, 4096, 3] (f32).
  dist[b,n,m] = ||p1[b,n]||^2 + ||p2[b,m]||^2 - 2 p1.p2
  loss = (mean_n,b(min_m dist) + mean_m,b(min_n dist)) / 8     (scalar f32)

Sharding: data-parallel over batch B: core b handles batch b.

v2 design (per core):
  Host: sort both clouds by x. Lift each point to K=16 fp16 rows
  (hi/lo split of every coordinate and squared norm, all cross terms)
  so that -d[n,m] = sum_k la[k,n]*lb[k,m] EXACTLY to ~1.5e-5: the
  matmul runs at bf16 speed (1 cyc/col) instead of fp32 (4 cyc/col).
  Windowing: strip i of 128 sorted points1 only computes distances to
  a W-wide contiguous window of sorted points2 centered at the same
  quantile. Cuts PE/ACT/DVE work by N/W.
  Device loop over 32 row-strips:
     PE:  4 matmuls (N=W/4, fp16, 4-way tile_position concurrency)
          -> PSUM strip [128, W] f32
     ACT: cast PSUM f32 -> SBUF fp16 strip
     DVE: colacc = max(colacc, strip) on the window (+ copy for newly
          covered columns); rowmax via fold-max tree W->128 (2 strips
          per op via 3D APs)
  Tail: per-strip rowmax fold 128->1 -> summ[:,0:32]; colacc partition
  max via 32 PE transposes + 4 tensor_reduce -> summ[:,32:64]; DMA the
  [128, 64] f32 summ out.
Host post: un-sort, certify each min against the window-edge bound
  (sound for any input); recompute the few uncertified points exactly
  on host; means -> loss.
"""

import sys
import numpy as np

for _p in ("/opt/trn_rl_repo", "/root/.axon_site/_ro/trn_rl_repo"):
    if _p not in sys.path:
        sys.path.insert(0, _p)

B = 8
N = 4096
D = 3
K = 16
P = 128
NSTRIP = N // P          # 32
W = 2048                 # window width (columns per strip)

_NC_CACHE = {}


def _window_lo(i, w=None):
    w = w or W
    return min(max(128 * i + 64 - w // 2, 0), N - w)


def _build_nc(repeat=1, w=W, parts="full"):
    import contextlib

    import concourse.bacc as bacc
    import concourse.tile as tile
    from concourse import mybir

    F16 = mybir.dt.float16
    F32 = mybir.dt.float32
    MAX = mybir.AluOpType.max

    do_act = parts != "mm"
    do_col = parts in ("mm+act+col", "full")
    do_row = parts in ("mm+act+row", "full")
    do_tail = parts == "full"

    MM_FREE = w // 4
    G = 4

    nc = bacc.Bacc(
        "TRN2", target_bir_lowering=False, debug=False, num_devices=B
    )
    la = nc.declare_dram_parameter("la", [K, N], F16, isOutput=False)
    lb = nc.declare_dram_parameter("lb", [K, N], F16, isOutput=False)
    ident = nc.declare_dram_parameter("ident", [P, P], F16, isOutput=False)
    out = nc.declare_dram_parameter("partial", [P, 2 * NSTRIP], F32, isOutput=True)

    with tile.TileContext(nc) as tc:
        with (
            tc.tile_pool(name="consts", bufs=1) as consts,
            tc.tile_pool(name="strips", bufs=2) as strips,
            tc.tile_pool(name="scr", bufs=2) as scr,
            tc.tile_pool(name="accs", bufs=1) as accs,
            tc.tile_pool(name="psum", bufs=2, space="PSUM") as psum,
        ):
            la_sb = consts.tile([3 * 32 + K, N], F16)
            lb_sb = consts.tile([3 * 32 + K, N], F16)
            for q in range(4):
                nc.sync.dma_start(out=la_sb[32 * q : 32 * q + K, :], in_=la[:])
                nc.scalar.dma_start(out=lb_sb[32 * q : 32 * q + K, :], in_=lb[:])
            idt = consts.tile([P, P], F16)
            nc.gpsimd.dma_start(out=idt[:], in_=ident[:])

            loop_ctx = (
                tc.For_i(0, repeat, 1) if repeat != 1 else contextlib.nullcontext()
            )
            with loop_ctx:
                colacc = accs.tile([P, N], F16)
                rowacc = accs.tile([P, NSTRIP * P], F16)
                summ = accs.tile([P, 2 * NSTRIP], F32)

                for ip in range(NSTRIP // G):
                    dstrip = strips.tile([P, G, w], F16, tag="strip")
                    for s in range(G):
                        i = G * ip + s
                        lo = _window_lo(i, w)
                        ph = psum.tile([P, w], F32, tag="ph")
                        for j in range(4):
                            m0 = j * MM_FREE
                            nc.tensor.matmul(
                                ph[:, m0 : m0 + MM_FREE],
                                lhsT=la_sb[
                                    32 * j : 32 * j + K, i * P : (i + 1) * P
                                ],
                                rhs=lb_sb[
                                    32 * j : 32 * j + K,
                                    lo + m0 : lo + m0 + MM_FREE,
                                ],
                                start=True,
                                stop=True,
                                tile_position=(32 * j, 0),
                            )
                        if do_act:
                            nc.scalar.copy(dstrip[:, s, :], ph[:])
                        if do_col:
                            # columns newly covered by this strip: copy;
                            # previously covered: elementwise max.
                            prev_hi = 0 if i == 0 else _window_lo(i - 1, w) + w
                            hi = lo + w
                            if i == 0:
                                nc.vector.tensor_copy(
                                    colacc[:, lo:hi], dstrip[:, s, :]
                                )
                            else:
                                if hi > prev_hi:
                                    nc.vector.tensor_copy(
                                        colacc[:, prev_hi:hi],
                                        dstrip[:, s, prev_hi - lo : w],
                                    )
                                nc.vector.tensor_tensor(
                                    colacc[:, lo:prev_hi],
                                    colacc[:, lo:prev_hi],
                                    dstrip[:, s, 0 : prev_hi - lo],
                                    op=MAX,
                                )
                        if do_row and s % 2 == 1:
                            s0 = s - 1
                            ww = w // 2
                            fsrc = dstrip[:, s0 : s0 + 2, :]
                            while ww > P:
                                fdst = scr.tile([P, 2, ww], F16, tag=f"fold{ww}")
                                nc.vector.tensor_tensor(
                                    fdst[:],
                                    fsrc[:, :, 0:ww],
                                    fsrc[:, :, ww : 2 * ww],
                                    op=MAX,
                                )
                                fsrc = fdst[:]
                                ww //= 2
                            i0 = G * ip + s0
                            nc.vector.tensor_tensor(
                                rowacc[:, i0 * P : (i0 + 2) * P].rearrange(
                                    "p (s q) -> p s q", s=2
                                ),
                                fsrc[:, :, 0:P],
                                fsrc[:, :, P : 2 * P],
                                op=MAX,
                            )

                # ---- tail ----
                if do_tail:
                    # per-strip rowmax: fold the 128 candidates to 2, reduce
                    rw = P // 2
                    v = rowacc[:].rearrange("p (i q) -> p i q", q=P)
                    while rw >= 2:
                        rdst = scr.tile([P, NSTRIP, rw], F16, tag=f"rfold{rw}")
                        nc.vector.tensor_tensor(
                            rdst[:], v[:, :, 0:rw], v[:, :, rw : 2 * rw], op=MAX
                        )
                        v = rdst[:]
                        rw //= 2
                    nc.vector.tensor_reduce(
                        out=summ[:, 0:NSTRIP],
                        in_=v,
                        axis=mybir.AxisListType.X,
                        op=MAX,
                    )
                    # colacc partition-max via PE transposes
                    tailp = psum.tile([P, NSTRIP, P], F16, tag="ph")
                    for q in range(4):
                        for t in range(8):
                            k = 8 * q + t
                            nc.tensor.transpose(
                                tailp[:, k, :],
                                colacc[:, k * P : (k + 1) * P],
                                idt[:],
                            )
                        nc.vector.tensor_reduce(
                            out=summ[:, NSTRIP + 8 * q : NSTRIP + 8 * q + 8],
                            in_=tailp[:, 8 * q : 8 * q + 8, :],
                            axis=mybir.AxisListType.X,
                            op=MAX,
                        )
                    nc.sync.dma_start(out=out[:], in_=summ[:])
                else:
                    zer = accs.tile([P, 2 * NSTRIP], F32)
                    nc.vector.memset(zer[:], 0.0)
                    nc.sync.dma_start(out=out[:], in_=zer[:])

    nc.compile()
    return nc


def get_nc(repeat=1, w=W, parts="full", **_ignored):
    key = (repeat, w, parts)
    if key not in _NC_CACHE:
        _NC_CACHE[key] = _build_nc(repeat=repeat, w=w, parts=parts)
    return _NC_CACHE[key]


def _f16(x):
    return x.astype(np.float16).astype(np.float32)


def _lift(p1, p2):
    """fp16 hi/lo lifted vectors (sorted clouds) so -dist = la^T @ lb.

    All 16 rows are exactly representable in fp16; the matmul in fp16
    with fp32 accumulate reproduces -d to ~1.5e-5 abs.
    """
    sq1 = (p1 * p1).sum(-1)
    sq2 = (p2 * p2).sum(-1)
    la = np.zeros((K, N), np.float32)
    lb = np.zeros((K, N), np.float32)
    s1h = _f16(sq1)
    s2h = _f16(sq2)
    la[0] = s1h
    lb[0] = -1.0
    la[1] = _f16(sq1 - s1h)
    lb[1] = -1.0
    la[2] = 1.0
    lb[2] = -s2h
    la[3] = 1.0
    lb[3] = -_f16(sq2 - s2h)
    for d in range(D):
        x = p1[:, d]
        y = p2[:, d]
        xh = _f16(x)
        xl = _f16(x - xh)
        yh = _f16(2.0 * y)
        yl = _f16(2.0 * y - yh)
        base = 4 + 4 * d
        la[base + 0] = xh
        lb[base + 0] = yh
        la[base + 1] = xh
        lb[base + 1] = yl
        la[base + 2] = xl
        lb[base + 2] = yh
        la[base + 3] = xl
        lb[base + 3] = yl
    return la.astype(np.float16), lb.astype(np.float16)


def _prep(points1, points2):
    """Sort by x per batch; return per-core input maps + sort state."""
    p1 = np.asarray(points1, dtype=np.float32)
    p2 = np.asarray(points2, dtype=np.float32)
    ident = np.eye(P, dtype=np.float16)
    in_maps = []
    state = []
    for b in range(B):
        o1 = np.argsort(p1[b, :, 0], kind="stable")
        o2 = np.argsort(p2[b, :, 0], kind="stable")
        s1 = p1[b][o1]
        s2 = p2[b][o2]
        la, lb = _lift(s1, s2)
        in_maps.append(
            {
                "la": np.ascontiguousarray(la),
                "lb": np.ascontiguousarray(lb),
                "ident": ident,
            }
        )
        state.append((s1, s2))
    return in_maps, state


def _in_maps(points1, points2):
    return _prep(points1, points2)[0]


def _postprocess(partial, s1, s2, w=W):
    """Un-sort device maxes, certify vs window-edge bounds, fix up.

    partial: [P, 64] f32 from one core (negated maxes).
    Returns (sum_min1, sum_min2) exact sums of per-point NN^2.
    """
    lo = np.array([_window_lo(i, w) for i in range(NSTRIP)])
    hi = lo + w
    # rowmins: sorted-p1 index n = 128*i + p  ->  partial[p, i]
    min1 = -partial[:, 0:NSTRIP].T.reshape(N).astype(np.float64)
    # colmins: sorted-p2 index m = 128*k + p  ->  partial[p, 32+k]
    min2 = -partial[:, NSTRIP : 2 * NSTRIP].T.reshape(N).astype(np.float64)

    x1 = s1[:, 0].astype(np.float64)
    x2 = s2[:, 0].astype(np.float64)

    # --- certify rowmins ---
    strip = np.arange(N) // P
    lo_n = lo[strip]
    hi_n = hi[strip]
    bound = np.full(N, np.inf)
    has_left = lo_n > 0
    gl = x1 - np.where(has_left, x2[np.maximum(lo_n - 1, 0)], -np.inf)
    bound = np.where(has_left, np.minimum(bound, np.maximum(gl, 0.0) ** 2), bound)
    has_right = hi_n < N
    gr = np.where(has_right, x2[np.minimum(hi_n, N - 1)], np.inf) - x1
    bound = np.where(
        has_right, np.minimum(bound, np.maximum(gr, 0.0) ** 2), bound
    )
    bad1 = np.nonzero(min1 * (1.0 + 2e-3) + 1e-6 >= bound)[0]
    for n in bad1:
        min1[n] = float(((s1[n] - s2) ** 2).sum(-1).min())

    # --- certify colmins ---
    # column m is covered by strips i with lo_i <= m < hi_i; covered rows
    # are a contiguous range [rlo_m, rhi_m).
    m = np.arange(N)
    # i_lo(m): first strip covering m = first i with hi_i > m
    ilo = np.searchsorted(hi, m, side="right")
    # i_hi(m): last strip covering m = last i with lo_i <= m
    ihi = np.searchsorted(lo, m, side="right") - 1
    rlo = ilo * P
    rhi = (ihi + 1) * P
    bound2 = np.full(N, np.inf)
    hasb = rlo > 0
    gb = x2 - np.where(hasb, x1[np.maximum(rlo - 1, 0)], -np.inf)
    bound2 = np.where(hasb, np.minimum(bound2, np.maximum(gb, 0.0) ** 2), bound2)
    hast = rhi < N
    gt = np.where(hast, x1[np.minimum(rhi, N - 1)], np.inf) - x2
    bound2 = np.where(
        hast, np.minimum(bound2, np.maximum(gt, 0.0) ** 2), bound2
    )
    bad2 = np.nonzero(min2 * (1.0 + 2e-3) + 1e-6 >= bound2)[0]
    for mm_ in bad2:
        min2[mm_] = float(((s2[mm_] - s1) ** 2).sum(-1).min())

    return min1.sum(), min2.sum()


def kernel(points1, points2):
    from concourse.bass_utils import run_bass_kernel_spmd

    in_maps, state = _prep(points1, points2)
    nc = get_nc()
    res = run_bass_kernel_spmd(nc, in_maps, list(range(B))).results
    tot = 0.0
    for b in range(B):
        s1, s2 = state[b]
        sum1, sum2 = _postprocess(np.asarray(res[b]["partial"]), s1, s2)
        tot += sum1 + sum2
    loss = tot / (B * N * B)
    return np.float32(loss)


# revision 64
# speedup vs baseline: 7.5109x; 3.1890x over previous
"""Chamfer distance loss kernel for 8 Trainium2 NeuronCores.

Problem: points1 [8, 4096, 3], points2 [8, 4096, 3] (f32).
  dist[b,n,m] = ||p1[b,n]||^2 + ||p2[b,m]||^2 - 2 p1.p2
  loss = (mean_n,b(min_m dist) + mean_m,b(min_n dist)) / 8     (scalar f32)

Sharding: data-parallel over batch B: core b handles batch b.

v2 design (per core):
  Host: sort both clouds by x. Lift each point to K=16 fp16 rows
  (hi/lo split of every coordinate and squared norm, all cross terms)
  so that -d[n,m] = sum_k la[k,n]*lb[k,m] exactly to ~1.5e-5: the
  matmul streams at 1 cyc/col (fp16) instead of fp32's 4 cyc/col.
  Windowing: strip i of 128 sorted points1 only computes distances to
  the W-wide contiguous window of sorted points2 at the same quantile.
  Cuts PE/ACT/DVE work by N/W (W=256 -> 16x fewer elements than the
  dense 4096^2 matrix).
  Device loop over 32 row-strips:
     PE:  matmul (free=W, fp16, K=16) -> PSUM [128, W] f32, rotating
          tile_position bands so LDWEIGHTS overlaps the previous matmul
     ACT: cast PSUM f32 -> SBUF fp16 strip
     DVE: colacc[:, window] = max(colacc, strip)  (colacc pre-set to
          -inf so every update is one uniform TT at fp16 2x mode);
          rowmax fold-max tree W->128 over G=4 strips per op (3D APs)
  Outputs (ship=3): colacc [128, 4096] f16 DMA'd out per quarter as
  soon as its columns are final (overlapped with the loop, 2 rings);
  rowacc folded 128->1 on DVE at the end -> partial [128, 32] f32.
Host post: un-sort; rowmins from partial, colmins = colacc.max(0);
  certify each min against the sorted-x window-edge bound (sound for
  any input); recompute the few uncertified points exactly on host;
  means -> loss.
"""

import sys
import numpy as np

for _p in ("/opt/trn_rl_repo", "/root/.axon_site/_ro/trn_rl_repo"):
    if _p not in sys.path:
        sys.path.insert(0, _p)

B = 8
N = 4096
D = 3
K = 16
P = 128
NSTRIP = N // P          # 32
W = 256                  # window width (columns per strip)

_NC_CACHE = {}


def _window_lo(i, w=None):
    w = w or W
    return min(max(128 * i + 64 - w // 2, 0), N - w)


def _build_nc(
    repeat=1, w=W, parts="full", nmm=None, midtail=0, g=4, paircast=0, ship=3,
    meminit="g",
):
    import contextlib

    import concourse.bacc as bacc
    import concourse.tile as tile
    from concourse import mybir

    F16 = mybir.dt.float16
    F32 = mybir.dt.float32
    MAX = mybir.AluOpType.max

    do_act = parts != "mm"
    do_col = parts in ("mm+act+col", "notail", "full")
    do_row = parts in ("mm+act+row", "notail", "full")
    do_tail = parts == "full"

    if nmm is None:
        nmm = max(1, w // 512)  # matmul PSUM writes must be 512-f32 (bank)
    MM_FREE = w // nmm
    G = g
    if do_tail and w > 1024 and not ship:
        # PSUM budget: 2 ph bufs (w*4 B) + 2 tailq bufs (2KB) must fit 16KB
        raise NotImplementedError("full tail only supported for w <= 1024")
    if parts != "full":
        ship = 0  # ablation variants use the plain partial output
    midtail = do_tail and midtail
    # PSUM: 16KB/partition total; tailq ring (2x2KB) reserved when tailing.
    # Matmul PSUM writes must start at a 2KB bank boundary, so each strip
    # gets a full [P, 512] f32 bank even when w < 512.
    pc = 2 if paircast else 1
    phw = max(w, 512)
    phbufs = max(2, min(8, (16384 - (4096 if do_tail else 0)) // (phw * 4 * pc)))
    # last strip whose window intersects colacc quarter q
    qdone = [
        max(
            i
            for i in range(NSTRIP)
            if _window_lo(i, w) < (N // 4) * (q + 1)
        )
        for q in range(4)
    ]

    nc = bacc.Bacc(
        "TRN2", target_bir_lowering=False, debug=False, num_devices=B
    )
    la = nc.declare_dram_parameter("la", [K, N], F16, isOutput=False)
    lb = nc.declare_dram_parameter("lb", [K, N], F16, isOutput=False)
    ident = nc.declare_dram_parameter("ident", [P, P], F16, isOutput=False)
    if ship in (1, 2):
        out_col = nc.declare_dram_parameter("colacc", [P, N], F16, isOutput=True)
        out_row = nc.declare_dram_parameter("rowacc", [P, N], F16, isOutput=True)
    elif ship == 3:
        out_col = nc.declare_dram_parameter("colacc", [P, N], F16, isOutput=True)
        out = nc.declare_dram_parameter("partial", [P, NSTRIP], F32, isOutput=True)
    else:
        out = nc.declare_dram_parameter(
            "partial", [P, 2 * NSTRIP], F32, isOutput=True
        )

    with tile.TileContext(nc) as tc:
        with (
            tc.tile_pool(name="consts", bufs=1) as consts,
            tc.tile_pool(name="strips", bufs=3) as strips,
            tc.tile_pool(name="scr", bufs=3) as scr,
            tc.tile_pool(name="accs", bufs=1) as accs,
            tc.tile_pool(name="psum", bufs=2, space="PSUM") as psum,
        ):
            la_sb = consts.tile([32 * 3 + K, N], F16)
            lb_sb = consts.tile([32 * 3 + K, N], F16)
            for q in range(4):
                nc.sync.dma_start(out=la_sb[32 * q : 32 * q + K, :], in_=la[:])
                nc.scalar.dma_start(out=lb_sb[32 * q : 32 * q + K, :], in_=lb[:])
            idt = consts.tile([P, P], F16)
            nc.gpsimd.dma_start(out=idt[:], in_=ident[:])

            colacc = accs.tile([P, N], F16)
            if do_col:
                # pre-init to -inf so every window update is one uniform
                # full-width max (gpsimd is otherwise idle)
                for q in range(4):
                    nc.gpsimd.memset(
                        colacc[:, q * (N // 4) : (q + 1) * (N // 4)], -60000.0
                    )

            loop_ctx = (
                tc.For_i(0, repeat, 1) if repeat != 1 else contextlib.nullcontext()
            )
            with loop_ctx:
                rowacc = accs.tile([P, NSTRIP * P], F16)
                summ = accs.tile([P, 2 * NSTRIP], F32)

                def emit_quarter(q):
                    tailq = psum.tile([P, 8, P], F16, tag="tailq")
                    for t in range(8):
                        k = 8 * q + t
                        nc.tensor.transpose(
                            tailq[:, t, :],
                            colacc[:, k * P : (k + 1) * P],
                            idt[:],
                        )
                    nc.vector.tensor_reduce(
                        out=summ[:, NSTRIP + 8 * q : NSTRIP + 8 * q + 8],
                        in_=tailq[:],
                        axis=mybir.AxisListType.X,
                        op=MAX,
                    )
                    # re-init for the next repeat iteration
                    me = nc.vector if meminit == "v" else nc.gpsimd
                    me.memset(
                        colacc[:, q * (N // 4) : (q + 1) * (N // 4)], -60000.0
                    )

                for ip in range(NSTRIP // G):
                    dstrip = strips.tile([P, G, w], F16, tag="strip")
                    ph_pair = None
                    for s in range(G):
                        i = G * ip + s
                        lo = _window_lo(i, w)
                        if pc == 1:
                            ph_full = psum.tile(
                                [P, phw], F32, tag="ph", bufs=phbufs
                            )
                            ph = ph_full[:, 0:w]
                        elif s % 2 == 0:
                            ph_pair = psum.tile(
                                [P, 2, phw], F32, tag="ph", bufs=phbufs
                            )
                            ph = ph_pair[:, 0, 0:w]
                        else:
                            ph = ph_pair[:, 1, 0:w]
                        for j in range(nmm):
                            # rotate tile-position bands across strips so
                            # LDWEIGHTS(i+1) overlaps MATMUL(i) (different
                            # row groups -> PE pulls the load ahead)
                            band = (i * nmm + j) % 4
                            m0 = j * MM_FREE
                            nc.tensor.matmul(
                                ph[:, m0 : m0 + MM_FREE],
                                lhsT=la_sb[
                                    32 * band : 32 * band + K,
                                    i * P : (i + 1) * P,
                                ],
                                rhs=lb_sb[
                                    32 * band : 32 * band + K,
                                    lo + m0 : lo + m0 + MM_FREE,
                                ],
                                start=True,
                                stop=True,
                                tile_position=(32 * band, 0),
                            )
                        if do_act and pc == 1:
                            nc.scalar.copy(dstrip[:, s, :], ph[:])
                        elif do_act and s % 2 == 1:
                            nc.scalar.copy(
                                dstrip[:, s - 1 : s + 1, :], ph_pair[:]
                            )
                        if do_col:
                            # colacc pre-set to -inf: one uniform window max
                            nc.vector.tensor_tensor(
                                colacc[:, lo : lo + w],
                                colacc[:, lo : lo + w],
                                dstrip[:, s, :],
                                op=MAX,
                            )
                        if do_tail and ship in (1, 3):
                            for q in range(4):
                                if qdone[q] == i:
                                    qs = slice(q * (N // 4), (q + 1) * (N // 4))
                                    eng = nc.sync if q % 2 == 0 else nc.gpsimd
                                    eng.dma_start(
                                        out=out_col[:, qs], in_=colacc[:, qs]
                                    )
                                    # re-init for the next repeat iteration
                                    me = nc.vector if meminit == "v" else nc.gpsimd
                                    me.memset(colacc[:, qs], -60000.0)
                        elif midtail:
                            for q in range(4):
                                if qdone[q] == i:
                                    emit_quarter(q)
                    if do_row:
                        v = w
                        fsrc = dstrip[:]
                        while v > 2 * P:
                            ww = v // 2
                            fdst = scr.tile([P, G, ww], F16, tag=f"fold{ww}")
                            nc.vector.tensor_tensor(
                                fdst[:],
                                fsrc[:, :, 0:ww],
                                fsrc[:, :, ww:v],
                                op=MAX,
                            )
                            fsrc = fdst[:]
                            v = ww
                        i0 = G * ip
                        # overlapped final halves (max is idempotent) so any
                        # 128 < v <= 256 reduces to the 128-wide rowacc slot
                        nc.vector.tensor_tensor(
                            rowacc[:, i0 * P : (i0 + G) * P].rearrange(
                                "p (s q) -> p s q", s=G
                            ),
                            fsrc[:, :, 0:P],
                            fsrc[:, :, v - P : v],
                            op=MAX,
                        )
                        if do_tail and ship == 1:
                            nc.gpsimd.dma_start(
                                out=out_row[:, i0 * P : (i0 + G) * P],
                                in_=rowacc[:, i0 * P : (i0 + G) * P],
                            )

                # ---- tail ----
                if do_tail and ship == 3:
                    rw = P // 2
                    v = rowacc[:].rearrange("p (i q) -> p i q", q=P)
                    while rw >= 2:
                        rdst = scr.tile([P, NSTRIP, rw], F16, tag=f"rfold{rw}")
                        nc.vector.tensor_tensor(
                            rdst[:], v[:, :, 0:rw], v[:, :, rw : 2 * rw], op=MAX
                        )
                        v = rdst[:]
                        rw //= 2
                    nc.vector.tensor_reduce(
                        out=summ[:, 0:NSTRIP],
                        in_=v,
                        axis=mybir.AxisListType.X,
                        op=MAX,
                    )
                    nc.sync.dma_start(out=out[:], in_=summ[:, 0:NSTRIP])
                elif do_tail and ship == 2:
                    for q in range(4):
                        nc.sync.dma_start(
                            out=out_col[:, q * (N // 4) : (q + 1) * (N // 4)],
                            in_=colacc[:, q * (N // 4) : (q + 1) * (N // 4)],
                        )
                        nc.gpsimd.dma_start(
                            out=out_row[:, q * (N // 4) : (q + 1) * (N // 4)],
                            in_=rowacc[:, q * (N // 4) : (q + 1) * (N // 4)],
                        )
                elif do_tail and ship:
                    pass  # accs already shipped incrementally
                elif do_tail:
                    if not midtail:
                        for q in range(4):
                            emit_quarter(q)
                    # per-strip rowmax: fold the 128 candidates to 2, reduce
                    rw = P // 2
                    v = rowacc[:].rearrange("p (i q) -> p i q", q=P)
                    while rw >= 2:
                        rdst = scr.tile([P, NSTRIP, rw], F16, tag=f"rfold{rw}")
                        nc.vector.tensor_tensor(
                            rdst[:], v[:, :, 0:rw], v[:, :, rw : 2 * rw], op=MAX
                        )
                        v = rdst[:]
                        rw //= 2
                    nc.vector.tensor_reduce(
                        out=summ[:, 0:NSTRIP],
                        in_=v,
                        axis=mybir.AxisListType.X,
                        op=MAX,
                    )
                    nc.sync.dma_start(out=out[:], in_=summ[:])
                elif not ship:
                    zer = accs.tile([P, 2 * NSTRIP], F32)
                    nc.vector.memset(zer[:], 0.0)
                    nc.sync.dma_start(out=out[:], in_=zer[:])

    nc.compile()
    return nc


def get_nc(
    repeat=1,
    w=W,
    parts="full",
    nmm=None,
    midtail=0,
    g=4,
    paircast=0,
    ship=3,
    meminit="g",
    **_ignored,
):
    key = (repeat, w, parts, nmm, midtail, g, paircast, ship, meminit)
    if key not in _NC_CACHE:
        _NC_CACHE[key] = _build_nc(
            repeat=repeat,
            w=w,
            parts=parts,
            nmm=nmm,
            midtail=midtail,
            g=g,
            paircast=paircast,
            ship=ship,
            meminit=meminit,
        )
    return _NC_CACHE[key]


def _f16(x):
    return x.astype(np.float16).astype(np.float32)


def _lift(p1, p2):
    """fp16 hi/lo lifted vectors (sorted clouds) so -dist = la^T @ lb.

    All 16 rows are exactly representable in fp16; the matmul in fp16
    with fp32 accumulate reproduces -d to ~1.5e-5 abs.
    """
    sq1 = (p1 * p1).sum(-1)
    sq2 = (p2 * p2).sum(-1)
    la = np.zeros((K, N), np.float32)
    lb = np.zeros((K, N), np.float32)
    s1h = _f16(sq1)
    s2h = _f16(sq2)
    la[0] = s1h
    lb[0] = -1.0
    la[1] = _f16(sq1 - s1h)
    lb[1] = -1.0
    la[2] = 1.0
    lb[2] = -s2h
    la[3] = 1.0
    lb[3] = -_f16(sq2 - s2h)
    for d in range(D):
        x = p1[:, d]
        y = p2[:, d]
        xh = _f16(x)
        xl = _f16(x - xh)
        yh = _f16(2.0 * y)
        yl = _f16(2.0 * y - yh)
        base = 4 + 4 * d
        la[base + 0] = xh
        lb[base + 0] = yh
        la[base + 1] = xh
        lb[base + 1] = yl
        la[base + 2] = xl
        lb[base + 2] = yh
        la[base + 3] = xl
        lb[base + 3] = yl
    return la.astype(np.float16), lb.astype(np.float16)


def _prep(points1, points2):
    """Sort by x per batch; return per-core input maps + sort state."""
    p1 = np.asarray(points1, dtype=np.float32)
    p2 = np.asarray(points2, dtype=np.float32)
    ident = np.eye(P, dtype=np.float16)
    in_maps = []
    state = []
    for b in range(B):
        o1 = np.argsort(p1[b, :, 0], kind="stable")
        o2 = np.argsort(p2[b, :, 0], kind="stable")
        s1 = p1[b][o1]
        s2 = p2[b][o2]
        la, lb = _lift(s1, s2)
        in_maps.append(
            {
                "la": np.ascontiguousarray(la),
                "lb": np.ascontiguousarray(lb),
                "ident": ident,
            }
        )
        state.append((s1, s2))
    return in_maps, state


def _in_maps(points1, points2):
    return _prep(points1, points2)[0]


def _postprocess(res, s1, s2, w=W):
    """Un-sort device maxes, certify vs window-edge bounds, fix up.

    res: one core's output dict (negated maxes).
    Returns (sum_min1, sum_min2) exact sums of per-point NN^2.
    """
    lo = np.array([_window_lo(i, w) for i in range(NSTRIP)])
    hi = lo + w
    if "partial" in res:
        partial = np.asarray(res["partial"])
        # rowmins: sorted-p1 index n = 128*i + p  ->  partial[p, i]
        min1 = -partial[:, 0:NSTRIP].T.reshape(N).astype(np.float64)
    else:
        # rowacc[p, i*128 + j]: candidate j for sorted-p1 index 128*i + p
        rowacc = np.asarray(res["rowacc"]).astype(np.float32)
        min1 = -(
            rowacc.reshape(P, NSTRIP, P).max(-1).T.reshape(N).astype(np.float64)
        )
    if "colacc" in res:
        # colacc[p, m]: strip-lane p candidate for sorted-p2 index m
        colacc = np.asarray(res["colacc"]).astype(np.float32)
        min2 = -(colacc.max(0).astype(np.float64))
    else:
        # colmins: sorted-p2 index m = 128*k + p  ->  partial[p, 32+k]
        min2 = -partial[:, NSTRIP : 2 * NSTRIP].T.reshape(N).astype(np.float64)

    x1 = s1[:, 0].astype(np.float64)
    x2 = s2[:, 0].astype(np.float64)

    # --- certify rowmins ---
    strip = np.arange(N) // P
    lo_n = lo[strip]
    hi_n = hi[strip]
    bound = np.full(N, np.inf)
    has_left = lo_n > 0
    gl = x1 - np.where(has_left, x2[np.maximum(lo_n - 1, 0)], -np.inf)
    bound = np.where(has_left, np.minimum(bound, np.maximum(gl, 0.0) ** 2), bound)
    has_right = hi_n < N
    gr = np.where(has_right, x2[np.minimum(hi_n, N - 1)], np.inf) - x1
    bound = np.where(
        has_right, np.minimum(bound, np.maximum(gr, 0.0) ** 2), bound
    )
    bad1 = np.nonzero(min1 * (1.0 + 1e-2) + 5e-5 >= bound)[0]
    if bad1.size:
        min1[bad1] = (
            ((s1[bad1, None, :].astype(np.float64) - s2[None, :, :]) ** 2)
            .sum(-1)
            .min(1)
        )

    # --- certify colmins ---
    # column m is covered by strips i with lo_i <= m < hi_i; covered rows
    # are a contiguous range [rlo_m, rhi_m).
    m = np.arange(N)
    # i_lo(m): first strip covering m = first i with hi_i > m
    ilo = np.searchsorted(hi, m, side="right")
    # i_hi(m): last strip covering m = last i with lo_i <= m
    ihi = np.searchsorted(lo, m, side="right") - 1
    rlo = ilo * P
    rhi = (ihi + 1) * P
    bound2 = np.full(N, np.inf)
    hasb = rlo > 0
    gb = x2 - np.where(hasb, x1[np.maximum(rlo - 1, 0)], -np.inf)
    bound2 = np.where(hasb, np.minimum(bound2, np.maximum(gb, 0.0) ** 2), bound2)
    hast = rhi < N
    gt = np.where(hast, x1[np.minimum(rhi, N - 1)], np.inf) - x2
    bound2 = np.where(
        hast, np.minimum(bound2, np.maximum(gt, 0.0) ** 2), bound2
    )
    bad2 = np.nonzero(min2 * (1.0 + 1e-2) + 5e-5 >= bound2)[0]
    if bad2.size:
        min2[bad2] = (
            ((s2[bad2, None, :].astype(np.float64) - s1[None, :, :]) ** 2)
            .sum(-1)
            .min(1)
        )

    return min1.sum(), min2.sum()


def kernel(points1, points2):
    from concourse.bass_utils import run_bass_kernel_spmd

    in_maps, state = _prep(points1, points2)
    nc = get_nc()
    res = run_bass_kernel_spmd(nc, in_maps, list(range(B))).results
    tot = 0.0
    for b in range(B):
        s1, s2 = state[b]
        sum1, sum2 = _postprocess(res[b], s1, s2)
        tot += sum1 + sum2
    loss = tot / (B * N * B)
    return np.float32(loss)


# revision 78
# speedup vs baseline: 8.3345x; 1.1097x over previous
"""Chamfer distance loss kernel for 8 Trainium2 NeuronCores.

Problem: points1 [8, 4096, 3], points2 [8, 4096, 3] (f32).
  dist[b,n,m] = ||p1[b,n]||^2 + ||p2[b,m]||^2 - 2 p1.p2
  loss = (mean_n,b(min_m dist) + mean_m,b(min_n dist)) / 8     (scalar f32)

Sharding: data-parallel over batch B: core b handles batch b.

v2 design (per core):
  Host: sort both clouds by x. Lift each point to K=16 fp16 rows
  (hi/lo split of every coordinate and squared norm, all cross terms)
  so that -d[n,m] = sum_k la[k,n]*lb[k,m] exactly to ~1.5e-5: the
  matmul streams at 1 cyc/col (fp16) instead of fp32's 4 cyc/col.
  Windowing: strip i of 128 sorted points1 only computes distances to
  the W-wide contiguous window of sorted points2 at the same quantile.
  Cuts PE/ACT/DVE work by N/W (W=256 -> 16x fewer elements than the
  dense 4096^2 matrix).
  Device loop over 32 row-strips:
     PE:  matmul (free=W, fp16, K=16) -> PSUM [128, W] f32, rotating
          tile_position bands so LDWEIGHTS overlaps the previous matmul
     ACT: cast PSUM f32 -> SBUF fp16 strip
     DVE: colacc[:, window] = max(colacc, strip)  (colacc pre-set to
          -inf so every update is one uniform TT at fp16 2x mode);
          rowmax fold-max tree W->128 over G=4 strips per op (3D APs)
  Outputs (ship=3): colacc [128, 4096] f16 DMA'd out per quarter as
  soon as its columns are final (overlapped with the loop, 2 rings);
  rowacc folded 128->1 on DVE at the end -> partial [128, 32] f32.
Host post: un-sort; rowmins from partial, colmins = colacc.max(0);
  certify each min against the sorted-x window-edge bound (sound for
  any input); recompute the few uncertified points exactly on host;
  means -> loss.
"""

import sys
import numpy as np

for _p in ("/opt/trn_rl_repo", "/root/.axon_site/_ro/trn_rl_repo"):
    if _p not in sys.path:
        sys.path.insert(0, _p)

B = 8
N = 4096
D = 3
K = 16
P = 128
NSTRIP = N // P          # 32
W = 256                  # window width (columns per strip)

_NC_CACHE = {}


def _window_lo(i, w=None):
    w = w or W
    return min(max(128 * i + 64 - w // 2, 0), N - w)


def _build_nc(
    repeat=1, w=W, parts="full", nmm=None, midtail=0, g=4, paircast=0, ship=3,
    meminit="v", rowx=0,
):
    import contextlib

    import concourse.bacc as bacc
    import concourse.tile as tile
    from concourse import mybir

    F16 = mybir.dt.float16
    F32 = mybir.dt.float32
    MAX = mybir.AluOpType.max

    do_act = parts != "mm"
    do_col = parts in ("mm+act+col", "notail", "full")
    do_row = parts in ("mm+act+row", "notail", "full")
    do_tail = parts == "full"

    if nmm is None:
        nmm = max(1, w // 512)  # matmul PSUM writes must be 512-f32 (bank)
    MM_FREE = w // nmm
    G = g
    if do_tail and w > 1024 and not ship:
        # PSUM budget: 2 ph bufs (w*4 B) + 2 tailq bufs (2KB) must fit 16KB
        raise NotImplementedError("full tail only supported for w <= 1024")
    if parts != "full":
        ship = 0  # ablation variants use the plain partial output
    midtail = do_tail and midtail
    # PSUM: 16KB/partition total; tailq ring (2x2KB) reserved when tailing.
    # Matmul PSUM writes must start at a 2KB bank boundary, so each strip
    # gets a full [P, 512] f32 bank even when w < 512.
    pc = 2 if paircast else 1
    phw = max(w, 512)
    phbufs = max(2, min(8, (16384 - (4096 if do_tail else 0)) // (phw * 4 * pc)))
    # last strip whose window intersects colacc quarter q
    qdone = [
        max(
            i
            for i in range(NSTRIP)
            if _window_lo(i, w) < (N // 4) * (q + 1)
        )
        for q in range(4)
    ]
    # last strip whose window intersects 128-col chunk c (for chunked ship)
    cdone = [
        max(i for i in range(NSTRIP) if _window_lo(i, w) < P * (c + 1))
        for c in range(NSTRIP)
    ]

    nc = bacc.Bacc(
        "TRN2", target_bir_lowering=False, debug=False, num_devices=B
    )
    la = nc.declare_dram_parameter("la", [K, N], F16, isOutput=False)
    lb = nc.declare_dram_parameter("lb", [K, N], F16, isOutput=False)
    ident = nc.declare_dram_parameter("ident", [P, P], F16, isOutput=False)
    if ship in (1, 2):
        out_col = nc.declare_dram_parameter("colacc", [P, N], F16, isOutput=True)
        out_row = nc.declare_dram_parameter("rowacc", [P, N], F16, isOutput=True)
    elif ship in (3, 9):
        out_col = nc.declare_dram_parameter("colacc", [P, N], F16, isOutput=True)
        out = nc.declare_dram_parameter("partial", [P, NSTRIP], F32, isOutput=True)
    else:
        out = nc.declare_dram_parameter(
            "partial", [P, 2 * NSTRIP], F32, isOutput=True
        )

    with tile.TileContext(nc) as tc:
        with (
            tc.tile_pool(name="consts", bufs=1) as consts,
            tc.tile_pool(name="strips", bufs=3) as strips,
            tc.tile_pool(name="scr", bufs=3) as scr,
            tc.tile_pool(name="accs", bufs=1) as accs,
            tc.tile_pool(name="psum", bufs=2, space="PSUM") as psum,
        ):
            la_sb = consts.tile([32 * 3 + K, N], F16)
            lb_sb = consts.tile([32 * 3 + K, N], F16)
            for q in range(4):
                nc.sync.dma_start(out=la_sb[32 * q : 32 * q + K, :], in_=la[:])
                nc.scalar.dma_start(out=lb_sb[32 * q : 32 * q + K, :], in_=lb[:])
            idt = consts.tile([P, P], F16)
            nc.gpsimd.dma_start(out=idt[:], in_=ident[:])

            # double-buffer per-iteration state in the repeat loop so the
            # ship DMAs / re-init of iteration k never serialize against
            # iteration k+1's accumulator writes (steady-state = single-shot)
            nbuf = 2 if (ship == 3 and do_tail and repeat > 1) else 1
            if nbuf == 2:
                assert repeat % 2 == 0, "repeat must be even for dbuf"
            colaccs, rowaccs, rowacc32s, summs = [], [], [], []
            for u in range(nbuf):
                colaccs.append(
                    accs.tile([P, N], F16, name=f"colacc{u}", tag=f"colacc{u}")
                )
                rowaccs.append(
                    accs.tile(
                        [P, NSTRIP * P], F16, name=f"rowacc{u}", tag=f"rowacc{u}"
                    )
                )
                rowacc32s.append(
                    accs.tile(
                        [P, NSTRIP * 32],
                        F16,
                        name=f"rowacc32{u}",
                        tag=f"rowacc32{u}",
                    )
                )
                summs.append(
                    accs.tile(
                        [P, 2 * NSTRIP], F32, name=f"summ{u}", tag=f"summ{u}"
                    )
                )
                if do_col:
                    # pre-init to -inf so every window update is one uniform
                    # full-width max (gpsimd is otherwise idle)
                    for q in range(4):
                        nc.gpsimd.memset(
                            colaccs[u][:, q * (N // 4) : (q + 1) * (N // 4)],
                            -60000.0,
                        )
            if ship == 9:  # hazard diagnostic: ship from a constant tile
                dummy = accs.tile([P, N], F16)
                nc.vector.memset(dummy[:], 0.0)

            def emit_iteration(colacc, rowacc, rowacc32, summ):
                def emit_quarter(q):
                    tailq = psum.tile([P, 8, P], F16, tag="tailq")
                    for t in range(8):
                        k = 8 * q + t
                        nc.tensor.transpose(
                            tailq[:, t, :],
                            colacc[:, k * P : (k + 1) * P],
                            idt[:],
                        )
                    nc.vector.tensor_reduce(
                        out=summ[:, NSTRIP + 8 * q : NSTRIP + 8 * q + 8],
                        in_=tailq[:],
                        axis=mybir.AxisListType.X,
                        op=MAX,
                    )
                    # re-init for the next repeat iteration
                    me = nc.vector if meminit == "v" else nc.gpsimd
                    me.memset(
                        colacc[:, q * (N // 4) : (q + 1) * (N // 4)], -60000.0
                    )

                for ip in range(NSTRIP // G):
                    dstrip = strips.tile([P, G, w], F16, tag="strip")
                    ph_pair = None
                    for s in range(G):
                        i = G * ip + s
                        lo = _window_lo(i, w)
                        if pc == 1:
                            ph_full = psum.tile(
                                [P, phw], F32, tag="ph", bufs=phbufs
                            )
                            ph = ph_full[:, 0:w]
                        elif s % 2 == 0:
                            ph_pair = psum.tile(
                                [P, 2, phw], F32, tag="ph", bufs=phbufs
                            )
                            ph = ph_pair[:, 0, 0:w]
                        else:
                            ph = ph_pair[:, 1, 0:w]
                        for j in range(nmm):
                            # rotate tile-position bands across strips so
                            # LDWEIGHTS(i+1) overlaps MATMUL(i) (different
                            # row groups -> PE pulls the load ahead)
                            band = (i * nmm + j) % 4
                            m0 = j * MM_FREE
                            nc.tensor.matmul(
                                ph[:, m0 : m0 + MM_FREE],
                                lhsT=la_sb[
                                    32 * band : 32 * band + K,
                                    i * P : (i + 1) * P,
                                ],
                                rhs=lb_sb[
                                    32 * band : 32 * band + K,
                                    lo + m0 : lo + m0 + MM_FREE,
                                ],
                                start=True,
                                stop=True,
                                tile_position=(32 * band, 0),
                            )
                        if do_act and pc == 1:
                            nc.scalar.copy(dstrip[:, s, :], ph[:])
                        elif do_act and s % 2 == 1:
                            nc.scalar.copy(
                                dstrip[:, s - 1 : s + 1, :], ph_pair[:]
                            )
                        if do_col:
                            # colacc pre-set to -inf: one uniform window max
                            nc.vector.tensor_tensor(
                                colacc[:, lo : lo + w],
                                colacc[:, lo : lo + w],
                                dstrip[:, s, :],
                                op=MAX,
                            )
                        if do_tail and ship in (1, 3, 9):
                            # ship each 128-col chunk the moment its columns
                            # are final so no bulk DMA is end-exposed
                            for c in range(NSTRIP):
                                if cdone[c] == i:
                                    cs = slice(c * P, (c + 1) * P)
                                    eng = nc.sync if c % 2 == 0 else nc.gpsimd
                                    src = dummy if ship == 9 else colacc
                                    eng.dma_start(
                                        out=out_col[:, cs], in_=src[:, cs]
                                    )
                            for q in range(4):
                                if qdone[q] == i:
                                    qs = slice(q * (N // 4), (q + 1) * (N // 4))
                                    # re-init for this buffer's next use
                                    me = (
                                        nc.vector
                                        if meminit == "v"
                                        else nc.gpsimd
                                    )
                                    me.memset(colacc[:, qs], -60000.0)
                        elif midtail:
                            for q in range(4):
                                if qdone[q] == i:
                                    emit_quarter(q)
                    if do_row:
                        v = w
                        fsrc = dstrip[:]
                        while v > 2 * P:
                            ww = v // 2
                            fdst = scr.tile([P, G, ww], F16, tag=f"fold{ww}")
                            nc.vector.tensor_tensor(
                                fdst[:],
                                fsrc[:, :, 0:ww],
                                fsrc[:, :, ww:v],
                                op=MAX,
                            )
                            fsrc = fdst[:]
                            v = ww
                        i0 = G * ip
                        # overlapped final halves (max is idempotent) so any
                        # 128 < v <= 256 reduces to the 128-wide rowacc slot
                        nc.vector.tensor_tensor(
                            rowacc[:, i0 * P : (i0 + G) * P].rearrange(
                                "p (s q) -> p s q", s=G
                            ),
                            fsrc[:, :, 0:P],
                            fsrc[:, :, v - P : v],
                            op=MAX,
                        )
                        if rowx:
                            # fold this group's rowmax candidates 128 -> 32
                            # in-loop (DVE slack) to shrink the end rfold
                            rsrc = rowacc[
                                :, i0 * P : (i0 + G) * P
                            ].rearrange("p (s q) -> p s q", s=G)
                            r64 = scr.tile([P, G, 64], F16, tag="rx64")
                            nc.vector.tensor_tensor(
                                r64[:], rsrc[:, :, 0:64], rsrc[:, :, 64:128],
                                op=MAX,
                            )
                            nc.vector.tensor_tensor(
                                rowacc32[:, i0 * 32 : (i0 + G) * 32].rearrange(
                                    "p (s q) -> p s q", s=G
                                ),
                                r64[:, :, 0:32],
                                r64[:, :, 32:64],
                                op=MAX,
                            )
                        if do_tail and ship == 1:
                            nc.gpsimd.dma_start(
                                out=out_row[:, i0 * P : (i0 + G) * P],
                                in_=rowacc[:, i0 * P : (i0 + G) * P],
                            )

                # ---- tail ----
                if do_tail and ship in (3, 9):
                    if rowx:
                        rw = 16
                        v = rowacc32[:].rearrange("p (i q) -> p i q", q=32)
                    else:
                        rw = P // 2
                        v = rowacc[:].rearrange("p (i q) -> p i q", q=P)
                    while rw >= 2:
                        rdst = scr.tile([P, NSTRIP, rw], F16, tag=f"rfold{rw}")
                        nc.vector.tensor_tensor(
                            rdst[:], v[:, :, 0:rw], v[:, :, rw : 2 * rw], op=MAX
                        )
                        v = rdst[:]
                        rw //= 2
                    nc.vector.tensor_reduce(
                        out=summ[:, 0:NSTRIP],
                        in_=v,
                        axis=mybir.AxisListType.X,
                        op=MAX,
                    )
                    nc.sync.dma_start(out=out[:], in_=summ[:, 0:NSTRIP])
                elif do_tail and ship == 2:
                    for q in range(4):
                        nc.sync.dma_start(
                            out=out_col[:, q * (N // 4) : (q + 1) * (N // 4)],
                            in_=colacc[:, q * (N // 4) : (q + 1) * (N // 4)],
                        )
                        nc.gpsimd.dma_start(
                            out=out_row[:, q * (N // 4) : (q + 1) * (N // 4)],
                            in_=rowacc[:, q * (N // 4) : (q + 1) * (N // 4)],
                        )
                elif do_tail:
                    if not midtail:
                        for q in range(4):
                            emit_quarter(q)
                    # per-strip rowmax: fold the 128 candidates to 2, reduce
                    rw = P // 2
                    v = rowacc[:].rearrange("p (i q) -> p i q", q=P)
                    while rw >= 2:
                        rdst = scr.tile([P, NSTRIP, rw], F16, tag=f"rfold{rw}")
                        nc.vector.tensor_tensor(
                            rdst[:], v[:, :, 0:rw], v[:, :, rw : 2 * rw], op=MAX
                        )
                        v = rdst[:]
                        rw //= 2
                    nc.vector.tensor_reduce(
                        out=summ[:, 0:NSTRIP],
                        in_=v,
                        axis=mybir.AxisListType.X,
                        op=MAX,
                    )
                    nc.sync.dma_start(out=out[:], in_=summ[:])
                elif not ship:
                    zer = accs.tile([P, 2 * NSTRIP], F32, tag="zer")
                    nc.vector.memset(zer[:], 0.0)
                    nc.sync.dma_start(out=out[:], in_=zer[:])

            if repeat == 1:
                emit_iteration(colaccs[0], rowaccs[0], rowacc32s[0], summs[0])
            else:
                with tc.For_i(0, repeat // nbuf, 1):
                    for u in range(nbuf):
                        emit_iteration(
                            colaccs[u], rowaccs[u], rowacc32s[u], summs[u]
                        )

    nc.compile()
    return nc


def get_nc(
    repeat=1,
    w=W,
    parts="full",
    nmm=None,
    midtail=0,
    g=4,
    paircast=0,
    ship=3,
    meminit="v",
    rowx=0,
    **_ignored,
):
    key = (repeat, w, parts, nmm, midtail, g, paircast, ship, meminit, rowx)
    if key not in _NC_CACHE:
        _NC_CACHE[key] = _build_nc(
            repeat=repeat,
            w=w,
            parts=parts,
            nmm=nmm,
            midtail=midtail,
            g=g,
            paircast=paircast,
            ship=ship,
            meminit=meminit,
            rowx=rowx,
        )
    return _NC_CACHE[key]


def _f16(x):
    return x.astype(np.float16).astype(np.float32)


def _lift(p1, p2):
    """fp16 hi/lo lifted vectors (sorted clouds) so -dist = la^T @ lb.

    All 16 rows are exactly representable in fp16; the matmul in fp16
    with fp32 accumulate reproduces -d to ~1.5e-5 abs.
    """
    sq1 = (p1 * p1).sum(-1)
    sq2 = (p2 * p2).sum(-1)
    la = np.zeros((K, N), np.float32)
    lb = np.zeros((K, N), np.float32)
    s1h = _f16(sq1)
    s2h = _f16(sq2)
    la[0] = s1h
    lb[0] = -1.0
    la[1] = _f16(sq1 - s1h)
    lb[1] = -1.0
    la[2] = 1.0
    lb[2] = -s2h
    la[3] = 1.0
    lb[3] = -_f16(sq2 - s2h)
    for d in range(D):
        x = p1[:, d]
        y = p2[:, d]
        xh = _f16(x)
        xl = _f16(x - xh)
        yh = _f16(2.0 * y)
        yl = _f16(2.0 * y - yh)
        base = 4 + 4 * d
        la[base + 0] = xh
        lb[base + 0] = yh
        la[base + 1] = xh
        lb[base + 1] = yl
        la[base + 2] = xl
        lb[base + 2] = yh
        la[base + 3] = xl
        lb[base + 3] = yl
    return la.astype(np.float16), lb.astype(np.float16)


def _prep(points1, points2):
    """Sort by x per batch; return per-core input maps + sort state."""
    p1 = np.asarray(points1, dtype=np.float32)
    p2 = np.asarray(points2, dtype=np.float32)
    ident = np.eye(P, dtype=np.float16)
    in_maps = []
    state = []
    for b in range(B):
        o1 = np.argsort(p1[b, :, 0], kind="stable")
        o2 = np.argsort(p2[b, :, 0], kind="stable")
        s1 = p1[b][o1]
        s2 = p2[b][o2]
        la, lb = _lift(s1, s2)
        in_maps.append(
            {
                "la": np.ascontiguousarray(la),
                "lb": np.ascontiguousarray(lb),
                "ident": ident,
            }
        )
        state.append((s1, s2))
    return in_maps, state


def _in_maps(points1, points2):
    return _prep(points1, points2)[0]


def _postprocess(res, s1, s2, w=W):
    """Un-sort device maxes, certify vs window-edge bounds, fix up.

    res: one core's output dict (negated maxes).
    Returns (sum_min1, sum_min2) exact sums of per-point NN^2.
    """
    lo = np.array([_window_lo(i, w) for i in range(NSTRIP)])
    hi = lo + w
    if "partial" in res:
        partial = np.asarray(res["partial"])
        # rowmins: sorted-p1 index n = 128*i + p  ->  partial[p, i]
        min1 = -partial[:, 0:NSTRIP].T.reshape(N).astype(np.float64)
    else:
        # rowacc[p, i*128 + j]: candidate j for sorted-p1 index 128*i + p
        rowacc = np.asarray(res["rowacc"]).astype(np.float32)
        min1 = -(
            rowacc.reshape(P, NSTRIP, P).max(-1).T.reshape(N).astype(np.float64)
        )
    if "colacc" in res:
        # colacc[p, m]: strip-lane p candidate for sorted-p2 index m
        colacc = np.asarray(res["colacc"]).astype(np.float32)
        min2 = -(colacc.max(0).astype(np.float64))
    else:
        # colmins: sorted-p2 index m = 128*k + p  ->  partial[p, 32+k]
        min2 = -partial[:, NSTRIP : 2 * NSTRIP].T.reshape(N).astype(np.float64)

    x1 = s1[:, 0].astype(np.float64)
    x2 = s2[:, 0].astype(np.float64)

    # --- certify rowmins ---
    strip = np.arange(N) // P
    lo_n = lo[strip]
    hi_n = hi[strip]
    bound = np.full(N, np.inf)
    has_left = lo_n > 0
    gl = x1 - np.where(has_left, x2[np.maximum(lo_n - 1, 0)], -np.inf)
    bound = np.where(has_left, np.minimum(bound, np.maximum(gl, 0.0) ** 2), bound)
    has_right = hi_n < N
    gr = np.where(has_right, x2[np.minimum(hi_n, N - 1)], np.inf) - x1
    bound = np.where(
        has_right, np.minimum(bound, np.maximum(gr, 0.0) ** 2), bound
    )
    bad1 = np.nonzero(min1 * (1.0 + 1e-2) + 5e-5 >= bound)[0]
    if bad1.size:
        min1[bad1] = (
            ((s1[bad1, None, :].astype(np.float64) - s2[None, :, :]) ** 2)
            .sum(-1)
            .min(1)
        )

    # --- certify colmins ---
    # column m is covered by strips i with lo_i <= m < hi_i; covered rows
    # are a contiguous range [rlo_m, rhi_m).
    m = np.arange(N)
    # i_lo(m): first strip covering m = first i with hi_i > m
    ilo = np.searchsorted(hi, m, side="right")
    # i_hi(m): last strip covering m = last i with lo_i <= m
    ihi = np.searchsorted(lo, m, side="right") - 1
    rlo = ilo * P
    rhi = (ihi + 1) * P
    bound2 = np.full(N, np.inf)
    hasb = rlo > 0
    gb = x2 - np.where(hasb, x1[np.maximum(rlo - 1, 0)], -np.inf)
    bound2 = np.where(hasb, np.minimum(bound2, np.maximum(gb, 0.0) ** 2), bound2)
    hast = rhi < N
    gt = np.where(hast, x1[np.minimum(rhi, N - 1)], np.inf) - x2
    bound2 = np.where(
        hast, np.minimum(bound2, np.maximum(gt, 0.0) ** 2), bound2
    )
    bad2 = np.nonzero(min2 * (1.0 + 1e-2) + 5e-5 >= bound2)[0]
    if bad2.size:
        min2[bad2] = (
            ((s2[bad2, None, :].astype(np.float64) - s1[None, :, :]) ** 2)
            .sum(-1)
            .min(1)
        )

    return min1.sum(), min2.sum()


def kernel(points1, points2):
    from concourse.bass_utils import run_bass_kernel_spmd

    in_maps, state = _prep(points1, points2)
    nc = get_nc()
    res = run_bass_kernel_spmd(nc, in_maps, list(range(B))).results
    tot = 0.0
    for b in range(B):
        s1, s2 = state[b]
        sum1, sum2 = _postprocess(res[b], s1, s2)
        tot += sum1 + sum2
    loss = tot / (B * N * B)
    return np.float32(loss)


# revision 82
# speedup vs baseline: 10.7600x; 1.2910x over previous
"""Chamfer distance loss kernel for 8 Trainium2 NeuronCores.

Problem: points1 [8, 4096, 3], points2 [8, 4096, 3] (f32).
  dist[b,n,m] = ||p1[b,n]||^2 + ||p2[b,m]||^2 - 2 p1.p2
  loss = (mean_n,b(min_m dist) + mean_m,b(min_n dist)) / 8     (scalar f32)

Sharding: data-parallel over batch B: core b handles batch b.

v2 design (per core):
  Host: sort both clouds by x. Lift each point to K=16 fp16 rows
  (hi/lo split of every coordinate and squared norm, all cross terms)
  so that -d[n,m] = sum_k la[k,n]*lb[k,m] exactly to ~1.5e-5: the
  matmul streams at 1 cyc/col (fp16) instead of fp32's 4 cyc/col.
  Windowing: strip i of 128 sorted points1 only computes distances to
  the W-wide contiguous window of sorted points2 at the same quantile.
  Cuts PE/ACT/DVE work by N/W (W=256 -> 16x fewer elements than the
  dense 4096^2 matrix).
  Device loop over 32 row-strips:
     PE:  matmul (free=W, fp16, K=16) -> PSUM [128, W] f32, rotating
          tile_position bands so LDWEIGHTS overlaps the previous matmul
     ACT: cast PSUM f32 -> SBUF fp16 strip
     DVE: colacc[:, window] = max(colacc, strip)  (colacc pre-set to
          -inf so every update is one uniform TT at fp16 2x mode);
          rowmax fold-max tree W->128 over G=4 strips per op (3D APs)
  Outputs (ship=3): colacc [128, 4096] f16 DMA'd out per quarter as
  soon as its columns are final (overlapped with the loop, 2 rings);
  rowacc folded 128->1 on DVE at the end -> partial [128, 32] f32.
Host post: un-sort; rowmins from partial, colmins = colacc.max(0);
  certify each min against the sorted-x window-edge bound (sound for
  any input); recompute the few uncertified points exactly on host;
  means -> loss.
"""

import sys
import numpy as np

for _p in ("/opt/trn_rl_repo", "/root/.axon_site/_ro/trn_rl_repo"):
    if _p not in sys.path:
        sys.path.insert(0, _p)

B = 8
N = 4096
D = 3
K = 16
P = 128
NSTRIP = N // P          # 32
W = 256                  # window width (columns per strip)

_NC_CACHE = {}


def _window_lo(i, w=None):
    w = w or W
    return min(max(128 * i + 64 - w // 2, 0), N - w)


def _build_nc(
    repeat=1, w=W, parts="full", nmm=None, midtail=0, g=4, paircast=0, ship=3,
    meminit="v", rowx=0,
):
    import contextlib

    import concourse.bacc as bacc
    import concourse.tile as tile
    from concourse import mybir

    F16 = mybir.dt.float16
    F32 = mybir.dt.float32
    MAX = mybir.AluOpType.max

    do_act = parts != "mm"
    do_col = parts in ("mm+act+col", "notail", "full")
    do_row = parts in ("mm+act+row", "notail", "full")
    do_tail = parts == "full"

    if nmm is None:
        nmm = max(1, w // 512)  # matmul PSUM writes must be 512-f32 (bank)
    MM_FREE = w // nmm
    G = g
    if do_tail and w > 1024 and not ship:
        # PSUM budget: 2 ph bufs (w*4 B) + 2 tailq bufs (2KB) must fit 16KB
        raise NotImplementedError("full tail only supported for w <= 1024")
    if parts != "full":
        ship = 0  # ablation variants use the plain partial output
    midtail = do_tail and midtail
    # PSUM: 16KB/partition total; tailq ring (2x2KB) reserved when tailing.
    # Matmul PSUM writes must start at a 2KB bank boundary, so each strip
    # gets a full [P, 512] f32 bank even when w < 512.
    pc = 2 if paircast else 1
    phw = max(w, 512)
    phbufs = max(2, min(8, (16384 - (4096 if do_tail else 0)) // (phw * 4 * pc)))
    # last strip whose window intersects colacc quarter q
    qdone = [
        max(
            i
            for i in range(NSTRIP)
            if _window_lo(i, w) < (N // 4) * (q + 1)
        )
        for q in range(4)
    ]
    # last strip whose window intersects 128-col chunk c (for chunked ship)
    cdone = [
        max(i for i in range(NSTRIP) if _window_lo(i, w) < P * (c + 1))
        for c in range(NSTRIP)
    ]
    # column spans with a constant contributor set: between consecutive
    # window breakpoints every column is covered by the same strips, so
    # the column max is a direct k-ary max of strip slices (no RMW chain)
    los = [_window_lo(i, w) for i in range(NSTRIP)]
    breaks = sorted({0, N, *los, *(l + w for l in los)})
    breaks = [b for b in breaks if 0 <= b <= N]
    spans = []  # (a, b, contributors, emit_strip)
    for a, b in zip(breaks[:-1], breaks[1:]):
        contrib = [i for i in range(NSTRIP) if los[i] <= a and b <= los[i] + w]
        assert contrib, f"uncovered span {a}:{b}"
        spans.append((a, b, contrib, max(contrib)))

    nc = bacc.Bacc(
        "TRN2", target_bir_lowering=False, debug=False, num_devices=B
    )
    la = nc.declare_dram_parameter("la", [K, N], F16, isOutput=False)
    lb = nc.declare_dram_parameter("lb", [K, N], F16, isOutput=False)
    ident = nc.declare_dram_parameter("ident", [P, P], F16, isOutput=False)
    if ship in (1, 2):
        out_col = nc.declare_dram_parameter("colacc", [P, N], F16, isOutput=True)
        out_row = nc.declare_dram_parameter("rowacc", [P, N], F16, isOutput=True)
    elif ship in (3, 9):
        out_col = nc.declare_dram_parameter("colacc", [P, N], F16, isOutput=True)
        out = nc.declare_dram_parameter("partial", [P, NSTRIP], F32, isOutput=True)
    else:
        out = nc.declare_dram_parameter(
            "partial", [P, 2 * NSTRIP], F32, isOutput=True
        )

    with tile.TileContext(nc) as tc:
        with (
            tc.tile_pool(name="consts", bufs=1) as consts,
            tc.tile_pool(name="strips", bufs=3) as strips,
            tc.tile_pool(name="scr", bufs=3) as scr,
            tc.tile_pool(name="accs", bufs=1) as accs,
            tc.tile_pool(name="psum", bufs=2, space="PSUM") as psum,
        ):
            la_sb = consts.tile([32 * 3 + K, N], F16)
            lb_sb = consts.tile([32 * 3 + K, N], F16)
            for q in range(4):
                nc.sync.dma_start(out=la_sb[32 * q : 32 * q + K, :], in_=la[:])
                nc.scalar.dma_start(out=lb_sb[32 * q : 32 * q + K, :], in_=lb[:])
            idt = consts.tile([P, P], F16)
            nc.gpsimd.dma_start(out=idt[:], in_=ident[:])

            # double-buffer per-iteration state in the repeat loop so the
            # ship DMAs / re-init of iteration k never serialize against
            # iteration k+1's accumulator writes (steady-state = single-shot)
            nbuf = 2 if (ship == 3 and do_tail and repeat > 1) else 1
            if nbuf == 2:
                assert repeat % 2 == 0, "repeat must be even for dbuf"
            colaccs, rowaccs, rowacc32s, summs = [], [], [], []
            for u in range(nbuf):
                colaccs.append(
                    accs.tile([P, N], F16, name=f"colacc{u}", tag=f"colacc{u}")
                )
                rowaccs.append(
                    accs.tile(
                        [P, NSTRIP * P], F16, name=f"rowacc{u}", tag=f"rowacc{u}"
                    )
                )
                rowacc32s.append(
                    accs.tile(
                        [P, NSTRIP * 32],
                        F16,
                        name=f"rowacc32{u}",
                        tag=f"rowacc32{u}",
                    )
                )
                summs.append(
                    accs.tile(
                        [P, 2 * NSTRIP], F32, name=f"summ{u}", tag=f"summ{u}"
                    )
                )
                if do_col and ship != 3:
                    # pre-init to -inf so every window update is one uniform
                    # full-width max (gpsimd is otherwise idle)
                    for q in range(4):
                        nc.gpsimd.memset(
                            colaccs[u][:, q * (N // 4) : (q + 1) * (N // 4)],
                            -60000.0,
                        )
            if ship == 9:  # hazard diagnostic: ship from a constant tile
                dummy = accs.tile([P, N], F16)
                nc.vector.memset(dummy[:], 0.0)

            def emit_iteration(colacc, rowacc, rowacc32, summ):
                def emit_quarter(q):
                    tailq = psum.tile([P, 8, P], F16, tag="tailq")
                    for t in range(8):
                        k = 8 * q + t
                        nc.tensor.transpose(
                            tailq[:, t, :],
                            colacc[:, k * P : (k + 1) * P],
                            idt[:],
                        )
                    nc.vector.tensor_reduce(
                        out=summ[:, NSTRIP + 8 * q : NSTRIP + 8 * q + 8],
                        in_=tailq[:],
                        axis=mybir.AxisListType.X,
                        op=MAX,
                    )
                    # re-init for the next repeat iteration
                    me = nc.vector if meminit == "v" else nc.gpsimd
                    me.memset(
                        colacc[:, q * (N // 4) : (q + 1) * (N // 4)], -60000.0
                    )

                strip_ap = {}
                for ip in range(NSTRIP // G):
                    dstrip = strips.tile([P, G, w], F16, tag="strip")
                    for _s in range(G):
                        strip_ap[G * ip + _s] = (dstrip, _s)
                    ph_pair = None
                    for s in range(G):
                        i = G * ip + s
                        lo = _window_lo(i, w)
                        if pc == 1:
                            ph_full = psum.tile(
                                [P, phw], F32, tag="ph", bufs=phbufs
                            )
                            ph = ph_full[:, 0:w]
                        elif s % 2 == 0:
                            ph_pair = psum.tile(
                                [P, 2, phw], F32, tag="ph", bufs=phbufs
                            )
                            ph = ph_pair[:, 0, 0:w]
                        else:
                            ph = ph_pair[:, 1, 0:w]
                        for j in range(nmm):
                            # rotate tile-position bands across strips so
                            # LDWEIGHTS(i+1) overlaps MATMUL(i) (different
                            # row groups -> PE pulls the load ahead)
                            band = (i * nmm + j) % 4
                            m0 = j * MM_FREE
                            nc.tensor.matmul(
                                ph[:, m0 : m0 + MM_FREE],
                                lhsT=la_sb[
                                    32 * band : 32 * band + K,
                                    i * P : (i + 1) * P,
                                ],
                                rhs=lb_sb[
                                    32 * band : 32 * band + K,
                                    lo + m0 : lo + m0 + MM_FREE,
                                ],
                                start=True,
                                stop=True,
                                tile_position=(32 * band, 0),
                            )
                        def colship(ii, ss):
                            llo = _window_lo(ii, w)
                            if do_col and ship == 3:
                                # write-only colacc: each span's column max is
                                # a direct k-ary max over its contributor
                                # strips' slices (no RMW chain, no memsets)
                                for sidx, (a, b, contrib, emit) in enumerate(
                                    spans
                                ):
                                    if emit != ii:
                                        continue
                                    srcs = []
                                    for ci in contrib:
                                        t, tsl = strip_ap[ci]
                                        off = a - los[ci]
                                        srcs.append(
                                            t[:, tsl, off : off + (b - a)]
                                        )
                                    if len(srcs) == 1:
                                        nc.vector.tensor_copy(
                                            colacc[:, a:b], srcs[0]
                                        )
                                    else:
                                        nc.vector.tensor_tensor(
                                            colacc[:, a:b],
                                            srcs[0],
                                            srcs[1],
                                            op=MAX,
                                        )
                                        for extra in srcs[2:]:
                                            nc.vector.tensor_tensor(
                                                colacc[:, a:b],
                                                colacc[:, a:b],
                                                extra,
                                                op=MAX,
                                            )
                                    eng = (
                                        nc.sync if sidx % 2 == 0 else nc.gpsimd
                                    )
                                    eng.dma_start(
                                        out=out_col[:, a:b],
                                        in_=colacc[:, a:b],
                                    )
                                return
                            if do_col:
                                # colacc pre-set to -inf: uniform window max
                                nc.vector.tensor_tensor(
                                    colacc[:, llo : llo + w],
                                    colacc[:, llo : llo + w],
                                    dstrip[:, ss, :],
                                    op=MAX,
                                )
                            if do_tail and ship in (1, 9):
                                # ship each 128-col chunk the moment its
                                # columns are final (no end-exposed DMA)
                                for c in range(NSTRIP):
                                    if cdone[c] == ii:
                                        cs = slice(c * P, (c + 1) * P)
                                        eng = (
                                            nc.sync
                                            if c % 2 == 0
                                            else nc.gpsimd
                                        )
                                        src = dummy if ship == 9 else colacc
                                        eng.dma_start(
                                            out=out_col[:, cs], in_=src[:, cs]
                                        )
                                for q in range(4):
                                    if qdone[q] == ii:
                                        qs = slice(
                                            q * (N // 4), (q + 1) * (N // 4)
                                        )
                                        # re-init for this buffer's next use
                                        me = (
                                            nc.vector
                                            if meminit == "v"
                                            else nc.gpsimd
                                        )
                                        me.memset(colacc[:, qs], -60000.0)
                            elif midtail:
                                for q in range(4):
                                    if qdone[q] == ii:
                                        emit_quarter(q)

                        if pc == 1:
                            if do_act:
                                nc.scalar.copy(dstrip[:, s, :], ph[:])
                            colship(i, s)
                        elif s % 2 == 1:
                            if do_act:
                                nc.scalar.copy(
                                    dstrip[:, s - 1 : s + 1, :],
                                    ph_pair[:, :, 0:w],
                                )
                            colship(i - 1, s - 1)
                            colship(i, s)
                    if do_row:
                        v = w
                        fsrc = dstrip[:]
                        while v > 2 * P:
                            ww = v // 2
                            fdst = scr.tile([P, G, ww], F16, tag=f"fold{ww}")
                            nc.vector.tensor_tensor(
                                fdst[:],
                                fsrc[:, :, 0:ww],
                                fsrc[:, :, ww:v],
                                op=MAX,
                            )
                            fsrc = fdst[:]
                            v = ww
                        i0 = G * ip
                        # overlapped final halves (max is idempotent) so any
                        # 128 < v <= 256 reduces to the 128-wide rowacc slot
                        nc.vector.tensor_tensor(
                            rowacc[:, i0 * P : (i0 + G) * P].rearrange(
                                "p (s q) -> p s q", s=G
                            ),
                            fsrc[:, :, 0:P],
                            fsrc[:, :, v - P : v],
                            op=MAX,
                        )
                        if rowx:
                            # fold this group's rowmax candidates 128 -> 32
                            # in-loop (DVE slack) to shrink the end rfold
                            rsrc = rowacc[
                                :, i0 * P : (i0 + G) * P
                            ].rearrange("p (s q) -> p s q", s=G)
                            r64 = scr.tile([P, G, 64], F16, tag="rx64")
                            nc.vector.tensor_tensor(
                                r64[:], rsrc[:, :, 0:64], rsrc[:, :, 64:128],
                                op=MAX,
                            )
                            nc.vector.tensor_tensor(
                                rowacc32[:, i0 * 32 : (i0 + G) * 32].rearrange(
                                    "p (s q) -> p s q", s=G
                                ),
                                r64[:, :, 0:32],
                                r64[:, :, 32:64],
                                op=MAX,
                            )
                        if do_tail and ship == 1:
                            nc.gpsimd.dma_start(
                                out=out_row[:, i0 * P : (i0 + G) * P],
                                in_=rowacc[:, i0 * P : (i0 + G) * P],
                            )

                # ---- tail ----
                if do_tail and ship in (3, 9):
                    if rowx:
                        rw = 16
                        v = rowacc32[:].rearrange("p (i q) -> p i q", q=32)
                    else:
                        rw = P // 2
                        v = rowacc[:].rearrange("p (i q) -> p i q", q=P)
                    while rw >= 2:
                        rdst = scr.tile([P, NSTRIP, rw], F16, tag=f"rfold{rw}")
                        nc.vector.tensor_tensor(
                            rdst[:], v[:, :, 0:rw], v[:, :, rw : 2 * rw], op=MAX
                        )
                        v = rdst[:]
                        rw //= 2
                    nc.vector.tensor_reduce(
                        out=summ[:, 0:NSTRIP],
                        in_=v,
                        axis=mybir.AxisListType.X,
                        op=MAX,
                    )
                    nc.sync.dma_start(out=out[:], in_=summ[:, 0:NSTRIP])
                elif do_tail and ship == 2:
                    for q in range(4):
                        nc.sync.dma_start(
                            out=out_col[:, q * (N // 4) : (q + 1) * (N // 4)],
                            in_=colacc[:, q * (N // 4) : (q + 1) * (N // 4)],
                        )
                        nc.gpsimd.dma_start(
                            out=out_row[:, q * (N // 4) : (q + 1) * (N // 4)],
                            in_=rowacc[:, q * (N // 4) : (q + 1) * (N // 4)],
                        )
                elif do_tail:
                    if not midtail:
                        for q in range(4):
                            emit_quarter(q)
                    # per-strip rowmax: fold the 128 candidates to 2, reduce
                    rw = P // 2
                    v = rowacc[:].rearrange("p (i q) -> p i q", q=P)
                    while rw >= 2:
                        rdst = scr.tile([P, NSTRIP, rw], F16, tag=f"rfold{rw}")
                        nc.vector.tensor_tensor(
                            rdst[:], v[:, :, 0:rw], v[:, :, rw : 2 * rw], op=MAX
                        )
                        v = rdst[:]
                        rw //= 2
                    nc.vector.tensor_reduce(
                        out=summ[:, 0:NSTRIP],
                        in_=v,
                        axis=mybir.AxisListType.X,
                        op=MAX,
                    )
                    nc.sync.dma_start(out=out[:], in_=summ[:])
                elif not ship:
                    zer = accs.tile([P, 2 * NSTRIP], F32, tag="zer")
                    nc.vector.memset(zer[:], 0.0)
                    nc.sync.dma_start(out=out[:], in_=zer[:])

            if repeat == 1:
                emit_iteration(colaccs[0], rowaccs[0], rowacc32s[0], summs[0])
            else:
                with tc.For_i(0, repeat // nbuf, 1):
                    for u in range(nbuf):
                        emit_iteration(
                            colaccs[u], rowaccs[u], rowacc32s[u], summs[u]
                        )

    nc.compile()
    return nc


def get_nc(
    repeat=1,
    w=W,
    parts="full",
    nmm=None,
    midtail=0,
    g=4,
    paircast=0,
    ship=3,
    meminit="v",
    rowx=0,
    **_ignored,
):
    key = (repeat, w, parts, nmm, midtail, g, paircast, ship, meminit, rowx)
    if key not in _NC_CACHE:
        _NC_CACHE[key] = _build_nc(
            repeat=repeat,
            w=w,
            parts=parts,
            nmm=nmm,
            midtail=midtail,
            g=g,
            paircast=paircast,
            ship=ship,
            meminit=meminit,
            rowx=rowx,
        )
    return _NC_CACHE[key]


def _f16(x):
    return x.astype(np.float16).astype(np.float32)


def _lift(p1, p2):
    """fp16 hi/lo lifted vectors (sorted clouds) so -dist = la^T @ lb.

    All 16 rows are exactly representable in fp16; the matmul in fp16
    with fp32 accumulate reproduces -d to ~1.5e-5 abs.
    """
    sq1 = (p1 * p1).sum(-1)
    sq2 = (p2 * p2).sum(-1)
    la = np.zeros((K, N), np.float32)
    lb = np.zeros((K, N), np.float32)
    s1h = _f16(sq1)
    s2h = _f16(sq2)
    la[0] = s1h
    lb[0] = -1.0
    la[1] = _f16(sq1 - s1h)
    lb[1] = -1.0
    la[2] = 1.0
    lb[2] = -s2h
    la[3] = 1.0
    lb[3] = -_f16(sq2 - s2h)
    for d in range(D):
        x = p1[:, d]
        y = p2[:, d]
        xh = _f16(x)
        xl = _f16(x - xh)
        yh = _f16(2.0 * y)
        yl = _f16(2.0 * y - yh)
        base = 4 + 4 * d
        la[base + 0] = xh
        lb[base + 0] = yh
        la[base + 1] = xh
        lb[base + 1] = yl
        la[base + 2] = xl
        lb[base + 2] = yh
        la[base + 3] = xl
        lb[base + 3] = yl
    return la.astype(np.float16), lb.astype(np.float16)


def _prep(points1, points2):
    """Sort by x per batch; return per-core input maps + sort state."""
    p1 = np.asarray(points1, dtype=np.float32)
    p2 = np.asarray(points2, dtype=np.float32)
    ident = np.eye(P, dtype=np.float16)
    in_maps = []
    state = []
    for b in range(B):
        o1 = np.argsort(p1[b, :, 0], kind="stable")
        o2 = np.argsort(p2[b, :, 0], kind="stable")
        s1 = p1[b][o1]
        s2 = p2[b][o2]
        la, lb = _lift(s1, s2)
        in_maps.append(
            {
                "la": np.ascontiguousarray(la),
                "lb": np.ascontiguousarray(lb),
                "ident": ident,
            }
        )
        state.append((s1, s2))
    return in_maps, state


def _in_maps(points1, points2):
    return _prep(points1, points2)[0]


def _postprocess(res, s1, s2, w=W):
    """Un-sort device maxes, certify vs window-edge bounds, fix up.

    res: one core's output dict (negated maxes).
    Returns (sum_min1, sum_min2) exact sums of per-point NN^2.
    """
    lo = np.array([_window_lo(i, w) for i in range(NSTRIP)])
    hi = lo + w
    if "partial" in res:
        partial = np.asarray(res["partial"])
        # rowmins: sorted-p1 index n = 128*i + p  ->  partial[p, i]
        min1 = -partial[:, 0:NSTRIP].T.reshape(N).astype(np.float64)
    else:
        # rowacc[p, i*128 + j]: candidate j for sorted-p1 index 128*i + p
        rowacc = np.asarray(res["rowacc"]).astype(np.float32)
        min1 = -(
            rowacc.reshape(P, NSTRIP, P).max(-1).T.reshape(N).astype(np.float64)
        )
    if "colacc" in res:
        # colacc[p, m]: strip-lane p candidate for sorted-p2 index m
        colacc = np.asarray(res["colacc"]).astype(np.float32)
        min2 = -(colacc.max(0).astype(np.float64))
    else:
        # colmins: sorted-p2 index m = 128*k + p  ->  partial[p, 32+k]
        min2 = -partial[:, NSTRIP : 2 * NSTRIP].T.reshape(N).astype(np.float64)

    x1 = s1[:, 0].astype(np.float64)
    x2 = s2[:, 0].astype(np.float64)

    # --- certify rowmins ---
    strip = np.arange(N) // P
    lo_n = lo[strip]
    hi_n = hi[strip]
    bound = np.full(N, np.inf)
    has_left = lo_n > 0
    gl = x1 - np.where(has_left, x2[np.maximum(lo_n - 1, 0)], -np.inf)
    bound = np.where(has_left, np.minimum(bound, np.maximum(gl, 0.0) ** 2), bound)
    has_right = hi_n < N
    gr = np.where(has_right, x2[np.minimum(hi_n, N - 1)], np.inf) - x1
    bound = np.where(
        has_right, np.minimum(bound, np.maximum(gr, 0.0) ** 2), bound
    )
    bad1 = np.nonzero(min1 * (1.0 + 1e-2) + 5e-5 >= bound)[0]
    if bad1.size:
        min1[bad1] = (
            ((s1[bad1, None, :].astype(np.float64) - s2[None, :, :]) ** 2)
            .sum(-1)
            .min(1)
        )

    # --- certify colmins ---
    # column m is covered by strips i with lo_i <= m < hi_i; covered rows
    # are a contiguous range [rlo_m, rhi_m).
    m = np.arange(N)
    # i_lo(m): first strip covering m = first i with hi_i > m
    ilo = np.searchsorted(hi, m, side="right")
    # i_hi(m): last strip covering m = last i with lo_i <= m
    ihi = np.searchsorted(lo, m, side="right") - 1
    rlo = ilo * P
    rhi = (ihi + 1) * P
    bound2 = np.full(N, np.inf)
    hasb = rlo > 0
    gb = x2 - np.where(hasb, x1[np.maximum(rlo - 1, 0)], -np.inf)
    bound2 = np.where(hasb, np.minimum(bound2, np.maximum(gb, 0.0) ** 2), bound2)
    hast = rhi < N
    gt = np.where(hast, x1[np.minimum(rhi, N - 1)], np.inf) - x2
    bound2 = np.where(
        hast, np.minimum(bound2, np.maximum(gt, 0.0) ** 2), bound2
    )
    bad2 = np.nonzero(min2 * (1.0 + 1e-2) + 5e-5 >= bound2)[0]
    if bad2.size:
        min2[bad2] = (
            ((s2[bad2, None, :].astype(np.float64) - s1[None, :, :]) ** 2)
            .sum(-1)
            .min(1)
        )

    return min1.sum(), min2.sum()


def kernel(points1, points2):
    from concourse.bass_utils import run_bass_kernel_spmd

    in_maps, state = _prep(points1, points2)
    nc = get_nc()
    res = run_bass_kernel_spmd(nc, in_maps, list(range(B))).results
    tot = 0.0
    for b in range(B):
        s1, s2 = state[b]
        sum1, sum2 = _postprocess(res[b], s1, s2)
        tot += sum1 + sum2
    loss = tot / (B * N * B)
    return np.float32(loss)


# revision 83
# speedup vs baseline: 11.2474x; 1.0453x over previous
"""Chamfer distance loss kernel for 8 Trainium2 NeuronCores.

Problem: points1 [8, 4096, 3], points2 [8, 4096, 3] (f32).
  dist[b,n,m] = ||p1[b,n]||^2 + ||p2[b,m]||^2 - 2 p1.p2
  loss = (mean_n,b(min_m dist) + mean_m,b(min_n dist)) / 8     (scalar f32)

Sharding: data-parallel over batch B: core b handles batch b.

v2 design (per core):
  Host: sort both clouds by x. Lift each point to K=16 fp16 rows
  (hi/lo split of every coordinate and squared norm, all cross terms)
  so that -d[n,m] = sum_k la[k,n]*lb[k,m] exactly to ~1.5e-5: the
  matmul streams at 1 cyc/col (fp16) instead of fp32's 4 cyc/col.
  Windowing: strip i of 128 sorted points1 only computes distances to
  the W-wide contiguous window of sorted points2 at the same quantile.
  Cuts PE/ACT/DVE work by N/W (W=256 -> 16x fewer elements than the
  dense 4096^2 matrix).
  Device loop over 32 row-strips:
     PE:  matmul (free=W, fp16, K=16) -> PSUM [128, W] f32, rotating
          tile_position bands so LDWEIGHTS overlaps the previous matmul
     ACT: cast PSUM f32 -> SBUF fp16 strip
     DVE: colacc[:, window] = max(colacc, strip)  (colacc pre-set to
          -inf so every update is one uniform TT at fp16 2x mode);
          rowmax fold-max tree W->128 over G=4 strips per op (3D APs)
  Outputs (ship=3): colacc [128, 4096] f16 DMA'd out per quarter as
  soon as its columns are final (overlapped with the loop, 2 rings);
  rowacc folded 128->1 on DVE at the end -> partial [128, 32] f32.
Host post: un-sort; rowmins from partial, colmins = colacc.max(0);
  certify each min against the sorted-x window-edge bound (sound for
  any input); recompute the few uncertified points exactly on host;
  means -> loss.
"""

import sys
import numpy as np

for _p in ("/opt/trn_rl_repo", "/root/.axon_site/_ro/trn_rl_repo"):
    if _p not in sys.path:
        sys.path.insert(0, _p)

B = 8
N = 4096
D = 3
K = 16
P = 128
NSTRIP = N // P          # 32
W = 256                  # window width (columns per strip)

_NC_CACHE = {}


def _window_lo(i, w=None):
    w = w or W
    return min(max(128 * i + 64 - w // 2, 0), N - w)


def _build_nc(
    repeat=1, w=W, parts="full", nmm=None, midtail=0, g=4, paircast=0, ship=3,
    meminit="v", rowx=0,
):
    import contextlib

    import concourse.bacc as bacc
    import concourse.tile as tile
    from concourse import mybir

    F16 = mybir.dt.float16
    F32 = mybir.dt.float32
    MAX = mybir.AluOpType.max

    do_act = parts != "mm"
    do_col = parts in ("mm+act+col", "notail", "full")
    do_row = parts in ("mm+act+row", "notail", "full")
    do_tail = parts == "full"

    if nmm is None:
        nmm = max(1, w // 512)  # matmul PSUM writes must be 512-f32 (bank)
    MM_FREE = w // nmm
    G = g
    if do_tail and w > 1024 and not ship:
        # PSUM budget: 2 ph bufs (w*4 B) + 2 tailq bufs (2KB) must fit 16KB
        raise NotImplementedError("full tail only supported for w <= 1024")
    if parts != "full":
        ship = 0  # ablation variants use the plain partial output
    midtail = do_tail and midtail
    # PSUM: 16KB/partition total; tailq ring (2x2KB) reserved when tailing.
    # Matmul PSUM writes must start at a 2KB bank boundary, so each strip
    # gets a full [P, 512] f32 bank even when w < 512.
    pc = 2 if paircast else 1
    phw = max(w, 512)
    phbufs = max(2, min(8, (16384 - (4096 if do_tail else 0)) // (phw * 4 * pc)))
    # last strip whose window intersects colacc quarter q
    qdone = [
        max(
            i
            for i in range(NSTRIP)
            if _window_lo(i, w) < (N // 4) * (q + 1)
        )
        for q in range(4)
    ]
    # last strip whose window intersects 128-col chunk c (for chunked ship)
    cdone = [
        max(i for i in range(NSTRIP) if _window_lo(i, w) < P * (c + 1))
        for c in range(NSTRIP)
    ]
    # column spans with a constant contributor set: between consecutive
    # window breakpoints every column is covered by the same strips, so
    # the column max is a direct k-ary max of strip slices (no RMW chain)
    los = [_window_lo(i, w) for i in range(NSTRIP)]
    breaks = sorted({0, N, *los, *(l + w for l in los)})
    breaks = [b for b in breaks if 0 <= b <= N]
    spans = []  # (a, b, contributors, emit_strip)
    for a, b in zip(breaks[:-1], breaks[1:]):
        contrib = [i for i in range(NSTRIP) if los[i] <= a and b <= los[i] + w]
        assert contrib, f"uncovered span {a}:{b}"
        spans.append((a, b, contrib, max(contrib)))

    nc = bacc.Bacc(
        "TRN2", target_bir_lowering=False, debug=False, num_devices=B
    )
    la = nc.declare_dram_parameter("la", [K, N], F16, isOutput=False)
    lb = nc.declare_dram_parameter("lb", [K, N], F16, isOutput=False)
    ident = nc.declare_dram_parameter("ident", [P, P], F16, isOutput=False)
    if ship in (1, 2, 4):
        out_col = nc.declare_dram_parameter("colacc", [P, N], F16, isOutput=True)
        out_row = nc.declare_dram_parameter("rowacc", [P, N], F16, isOutput=True)
    elif ship in (3, 9):
        out_col = nc.declare_dram_parameter("colacc", [P, N], F16, isOutput=True)
        out = nc.declare_dram_parameter("partial", [P, NSTRIP], F32, isOutput=True)
    else:
        out = nc.declare_dram_parameter(
            "partial", [P, 2 * NSTRIP], F32, isOutput=True
        )

    with tile.TileContext(nc) as tc:
        with (
            tc.tile_pool(name="consts", bufs=1) as consts,
            tc.tile_pool(name="strips", bufs=3) as strips,
            tc.tile_pool(name="scr", bufs=3) as scr,
            tc.tile_pool(name="accs", bufs=1) as accs,
            tc.tile_pool(name="psum", bufs=2, space="PSUM") as psum,
        ):
            la_sb = consts.tile([32 * 3 + K, N], F16)
            lb_sb = consts.tile([32 * 3 + K, N], F16)
            for q in range(4):
                nc.sync.dma_start(out=la_sb[32 * q : 32 * q + K, :], in_=la[:])
                nc.scalar.dma_start(out=lb_sb[32 * q : 32 * q + K, :], in_=lb[:])
            idt = consts.tile([P, P], F16)
            nc.gpsimd.dma_start(out=idt[:], in_=ident[:])

            # double-buffer per-iteration state in the repeat loop so the
            # ship DMAs / re-init of iteration k never serialize against
            # iteration k+1's accumulator writes (steady-state = single-shot)
            nbuf = 2 if (ship in (3, 4) and do_tail and repeat > 1) else 1
            if nbuf == 2:
                assert repeat % 2 == 0, "repeat must be even for dbuf"
            colaccs, rowaccs, rowacc32s, summs = [], [], [], []
            for u in range(nbuf):
                colaccs.append(
                    accs.tile([P, N], F16, name=f"colacc{u}", tag=f"colacc{u}")
                )
                rowaccs.append(
                    accs.tile(
                        [P, NSTRIP * P], F16, name=f"rowacc{u}", tag=f"rowacc{u}"
                    )
                )
                rowacc32s.append(
                    accs.tile(
                        [P, NSTRIP * 32],
                        F16,
                        name=f"rowacc32{u}",
                        tag=f"rowacc32{u}",
                    )
                )
                summs.append(
                    accs.tile(
                        [P, 2 * NSTRIP], F32, name=f"summ{u}", tag=f"summ{u}"
                    )
                )
                if do_col and ship not in (3, 4):
                    # pre-init to -inf so every window update is one uniform
                    # full-width max (gpsimd is otherwise idle)
                    for q in range(4):
                        nc.gpsimd.memset(
                            colaccs[u][:, q * (N // 4) : (q + 1) * (N // 4)],
                            -60000.0,
                        )
            if ship == 9:  # hazard diagnostic: ship from a constant tile
                dummy = accs.tile([P, N], F16)
                nc.vector.memset(dummy[:], 0.0)

            def emit_iteration(colacc, rowacc, rowacc32, summ):
                def emit_quarter(q):
                    tailq = psum.tile([P, 8, P], F16, tag="tailq")
                    for t in range(8):
                        k = 8 * q + t
                        nc.tensor.transpose(
                            tailq[:, t, :],
                            colacc[:, k * P : (k + 1) * P],
                            idt[:],
                        )
                    nc.vector.tensor_reduce(
                        out=summ[:, NSTRIP + 8 * q : NSTRIP + 8 * q + 8],
                        in_=tailq[:],
                        axis=mybir.AxisListType.X,
                        op=MAX,
                    )
                    # re-init for the next repeat iteration
                    me = nc.vector if meminit == "v" else nc.gpsimd
                    me.memset(
                        colacc[:, q * (N // 4) : (q + 1) * (N // 4)], -60000.0
                    )

                strip_ap = {}
                for ip in range(NSTRIP // G):
                    dstrip = strips.tile([P, G, w], F16, tag="strip")
                    for _s in range(G):
                        strip_ap[G * ip + _s] = (dstrip, _s)
                    ph_pair = None
                    for s in range(G):
                        i = G * ip + s
                        lo = _window_lo(i, w)
                        if pc == 1:
                            ph_full = psum.tile(
                                [P, phw], F32, tag="ph", bufs=phbufs
                            )
                            ph = ph_full[:, 0:w]
                        elif s % 2 == 0:
                            ph_pair = psum.tile(
                                [P, 2, phw], F32, tag="ph", bufs=phbufs
                            )
                            ph = ph_pair[:, 0, 0:w]
                        else:
                            ph = ph_pair[:, 1, 0:w]
                        for j in range(nmm):
                            # rotate tile-position bands across strips so
                            # LDWEIGHTS(i+1) overlaps MATMUL(i) (different
                            # row groups -> PE pulls the load ahead)
                            band = (i * nmm + j) % 4
                            m0 = j * MM_FREE
                            nc.tensor.matmul(
                                ph[:, m0 : m0 + MM_FREE],
                                lhsT=la_sb[
                                    32 * band : 32 * band + K,
                                    i * P : (i + 1) * P,
                                ],
                                rhs=lb_sb[
                                    32 * band : 32 * band + K,
                                    lo + m0 : lo + m0 + MM_FREE,
                                ],
                                start=True,
                                stop=True,
                                tile_position=(32 * band, 0),
                            )
                        def colship(ii, ss):
                            llo = _window_lo(ii, w)
                            if do_col and ship in (3, 4):
                                # write-only colacc: each span's column max is
                                # a direct k-ary max over its contributor
                                # strips' slices (no RMW chain, no memsets)
                                for sidx, (a, b, contrib, emit) in enumerate(
                                    spans
                                ):
                                    if emit != ii:
                                        continue
                                    srcs = []
                                    for ci in contrib:
                                        t, tsl = strip_ap[ci]
                                        off = a - los[ci]
                                        srcs.append(
                                            t[:, tsl, off : off + (b - a)]
                                        )
                                    if len(srcs) == 1:
                                        nc.vector.tensor_copy(
                                            colacc[:, a:b], srcs[0]
                                        )
                                    else:
                                        nc.vector.tensor_tensor(
                                            colacc[:, a:b],
                                            srcs[0],
                                            srcs[1],
                                            op=MAX,
                                        )
                                        for extra in srcs[2:]:
                                            nc.vector.tensor_tensor(
                                                colacc[:, a:b],
                                                colacc[:, a:b],
                                                extra,
                                                op=MAX,
                                            )
                                    eng = (
                                        nc.sync if sidx % 2 == 0 else nc.gpsimd
                                    )
                                    eng.dma_start(
                                        out=out_col[:, a:b],
                                        in_=colacc[:, a:b],
                                    )
                                return
                            if do_col:
                                # colacc pre-set to -inf: uniform window max
                                nc.vector.tensor_tensor(
                                    colacc[:, llo : llo + w],
                                    colacc[:, llo : llo + w],
                                    dstrip[:, ss, :],
                                    op=MAX,
                                )
                            if do_tail and ship in (1, 9):
                                # ship each 128-col chunk the moment its
                                # columns are final (no end-exposed DMA)
                                for c in range(NSTRIP):
                                    if cdone[c] == ii:
                                        cs = slice(c * P, (c + 1) * P)
                                        eng = (
                                            nc.sync
                                            if c % 2 == 0
                                            else nc.gpsimd
                                        )
                                        src = dummy if ship == 9 else colacc
                                        eng.dma_start(
                                            out=out_col[:, cs], in_=src[:, cs]
                                        )
                                for q in range(4):
                                    if qdone[q] == ii:
                                        qs = slice(
                                            q * (N // 4), (q + 1) * (N // 4)
                                        )
                                        # re-init for this buffer's next use
                                        me = (
                                            nc.vector
                                            if meminit == "v"
                                            else nc.gpsimd
                                        )
                                        me.memset(colacc[:, qs], -60000.0)
                            elif midtail:
                                for q in range(4):
                                    if qdone[q] == ii:
                                        emit_quarter(q)

                        if pc == 1:
                            if do_act:
                                nc.scalar.copy(dstrip[:, s, :], ph[:])
                            colship(i, s)
                        elif s % 2 == 1:
                            if do_act:
                                nc.scalar.copy(
                                    dstrip[:, s - 1 : s + 1, :],
                                    ph_pair[:, :, 0:w],
                                )
                            colship(i - 1, s - 1)
                            colship(i, s)
                    if do_row:
                        v = w
                        fsrc = dstrip[:]
                        while v > 2 * P:
                            ww = v // 2
                            fdst = scr.tile([P, G, ww], F16, tag=f"fold{ww}")
                            nc.vector.tensor_tensor(
                                fdst[:],
                                fsrc[:, :, 0:ww],
                                fsrc[:, :, ww:v],
                                op=MAX,
                            )
                            fsrc = fdst[:]
                            v = ww
                        i0 = G * ip
                        # overlapped final halves (max is idempotent) so any
                        # 128 < v <= 256 reduces to the 128-wide rowacc slot
                        nc.vector.tensor_tensor(
                            rowacc[:, i0 * P : (i0 + G) * P].rearrange(
                                "p (s q) -> p s q", s=G
                            ),
                            fsrc[:, :, 0:P],
                            fsrc[:, :, v - P : v],
                            op=MAX,
                        )
                        if rowx:
                            # fold this group's rowmax candidates 128 -> 32
                            # in-loop (DVE slack) to shrink the end rfold
                            rsrc = rowacc[
                                :, i0 * P : (i0 + G) * P
                            ].rearrange("p (s q) -> p s q", s=G)
                            r64 = scr.tile([P, G, 64], F16, tag="rx64")
                            nc.vector.tensor_tensor(
                                r64[:], rsrc[:, :, 0:64], rsrc[:, :, 64:128],
                                op=MAX,
                            )
                            nc.vector.tensor_tensor(
                                rowacc32[:, i0 * 32 : (i0 + G) * 32].rearrange(
                                    "p (s q) -> p s q", s=G
                                ),
                                r64[:, :, 0:32],
                                r64[:, :, 32:64],
                                op=MAX,
                            )
                        if do_tail and ship == 1:
                            nc.gpsimd.dma_start(
                                out=out_row[:, i0 * P : (i0 + G) * P],
                                in_=rowacc[:, i0 * P : (i0 + G) * P],
                            )
                        elif do_tail and ship == 4:
                            # raw rowmax candidates out on the ACT-issued
                            # ring; host does the 128-way fold
                            nc.scalar.dma_start(
                                out=out_row[:, i0 * P : (i0 + G) * P],
                                in_=rowacc[:, i0 * P : (i0 + G) * P],
                            )

                # ---- tail ----
                if do_tail and ship == 4:
                    pass  # everything shipped incrementally
                elif do_tail and ship in (3, 9):
                    if rowx:
                        rw = 16
                        v = rowacc32[:].rearrange("p (i q) -> p i q", q=32)
                    else:
                        rw = P // 2
                        v = rowacc[:].rearrange("p (i q) -> p i q", q=P)
                    while rw >= 2:
                        rdst = scr.tile([P, NSTRIP, rw], F16, tag=f"rfold{rw}")
                        nc.vector.tensor_tensor(
                            rdst[:], v[:, :, 0:rw], v[:, :, rw : 2 * rw], op=MAX
                        )
                        v = rdst[:]
                        rw //= 2
                    nc.vector.tensor_reduce(
                        out=summ[:, 0:NSTRIP],
                        in_=v,
                        axis=mybir.AxisListType.X,
                        op=MAX,
                    )
                    nc.sync.dma_start(out=out[:], in_=summ[:, 0:NSTRIP])
                elif do_tail and ship == 2:
                    for q in range(4):
                        nc.sync.dma_start(
                            out=out_col[:, q * (N // 4) : (q + 1) * (N // 4)],
                            in_=colacc[:, q * (N // 4) : (q + 1) * (N // 4)],
                        )
                        nc.gpsimd.dma_start(
                            out=out_row[:, q * (N // 4) : (q + 1) * (N // 4)],
                            in_=rowacc[:, q * (N // 4) : (q + 1) * (N // 4)],
                        )
                elif do_tail:
                    if not midtail:
                        for q in range(4):
                            emit_quarter(q)
                    # per-strip rowmax: fold the 128 candidates to 2, reduce
                    rw = P // 2
                    v = rowacc[:].rearrange("p (i q) -> p i q", q=P)
                    while rw >= 2:
                        rdst = scr.tile([P, NSTRIP, rw], F16, tag=f"rfold{rw}")
                        nc.vector.tensor_tensor(
                            rdst[:], v[:, :, 0:rw], v[:, :, rw : 2 * rw], op=MAX
                        )
                        v = rdst[:]
                        rw //= 2
                    nc.vector.tensor_reduce(
                        out=summ[:, 0:NSTRIP],
                        in_=v,
                        axis=mybir.AxisListType.X,
                        op=MAX,
                    )
                    nc.sync.dma_start(out=out[:], in_=summ[:])
                elif not ship:
                    zer = accs.tile([P, 2 * NSTRIP], F32, tag="zer")
                    nc.vector.memset(zer[:], 0.0)
                    nc.sync.dma_start(out=out[:], in_=zer[:])

            if repeat == 1:
                emit_iteration(colaccs[0], rowaccs[0], rowacc32s[0], summs[0])
            else:
                with tc.For_i(0, repeat // nbuf, 1):
                    for u in range(nbuf):
                        emit_iteration(
                            colaccs[u], rowaccs[u], rowacc32s[u], summs[u]
                        )

    nc.compile()
    return nc


def get_nc(
    repeat=1,
    w=W,
    parts="full",
    nmm=None,
    midtail=0,
    g=4,
    paircast=0,
    ship=3,
    meminit="v",
    rowx=0,
    **_ignored,
):
    key = (repeat, w, parts, nmm, midtail, g, paircast, ship, meminit, rowx)
    if key not in _NC_CACHE:
        _NC_CACHE[key] = _build_nc(
            repeat=repeat,
            w=w,
            parts=parts,
            nmm=nmm,
            midtail=midtail,
            g=g,
            paircast=paircast,
            ship=ship,
            meminit=meminit,
            rowx=rowx,
        )
    return _NC_CACHE[key]


def _f16(x):
    return x.astype(np.float16).astype(np.float32)


def _lift(p1, p2):
    """fp16 hi/lo lifted vectors (sorted clouds) so -dist = la^T @ lb.

    All 16 rows are exactly representable in fp16; the matmul in fp16
    with fp32 accumulate reproduces -d to ~1.5e-5 abs.
    """
    sq1 = (p1 * p1).sum(-1)
    sq2 = (p2 * p2).sum(-1)
    la = np.zeros((K, N), np.float32)
    lb = np.zeros((K, N), np.float32)
    s1h = _f16(sq1)
    s2h = _f16(sq2)
    la[0] = s1h
    lb[0] = -1.0
    la[1] = _f16(sq1 - s1h)
    lb[1] = -1.0
    la[2] = 1.0
    lb[2] = -s2h
    la[3] = 1.0
    lb[3] = -_f16(sq2 - s2h)
    for d in range(D):
        x = p1[:, d]
        y = p2[:, d]
        xh = _f16(x)
        xl = _f16(x - xh)
        yh = _f16(2.0 * y)
        yl = _f16(2.0 * y - yh)
        base = 4 + 4 * d
        la[base + 0] = xh
        lb[base + 0] = yh
        la[base + 1] = xh
        lb[base + 1] = yl
        la[base + 2] = xl
        lb[base + 2] = yh
        la[base + 3] = xl
        lb[base + 3] = yl
    return la.astype(np.float16), lb.astype(np.float16)


def _prep(points1, points2):
    """Sort by x per batch; return per-core input maps + sort state."""
    p1 = np.asarray(points1, dtype=np.float32)
    p2 = np.asarray(points2, dtype=np.float32)
    ident = np.eye(P, dtype=np.float16)
    in_maps = []
    state = []
    for b in range(B):
        o1 = np.argsort(p1[b, :, 0], kind="stable")
        o2 = np.argsort(p2[b, :, 0], kind="stable")
        s1 = p1[b][o1]
        s2 = p2[b][o2]
        la, lb = _lift(s1, s2)
        in_maps.append(
            {
                "la": np.ascontiguousarray(la),
                "lb": np.ascontiguousarray(lb),
                "ident": ident,
            }
        )
        state.append((s1, s2))
    return in_maps, state


def _in_maps(points1, points2):
    return _prep(points1, points2)[0]


def _postprocess(res, s1, s2, w=W):
    """Un-sort device maxes, certify vs window-edge bounds, fix up.

    res: one core's output dict (negated maxes).
    Returns (sum_min1, sum_min2) exact sums of per-point NN^2.
    """
    lo = np.array([_window_lo(i, w) for i in range(NSTRIP)])
    hi = lo + w
    if "partial" in res:
        partial = np.asarray(res["partial"])
        # rowmins: sorted-p1 index n = 128*i + p  ->  partial[p, i]
        min1 = -partial[:, 0:NSTRIP].T.reshape(N).astype(np.float64)
    else:
        # rowacc[p, i*128 + j]: candidate j for sorted-p1 index 128*i + p
        rowacc = np.asarray(res["rowacc"]).astype(np.float32)
        min1 = -(
            rowacc.reshape(P, NSTRIP, P).max(-1).T.reshape(N).astype(np.float64)
        )
    if "colacc" in res:
        # colacc[p, m]: strip-lane p candidate for sorted-p2 index m
        colacc = np.asarray(res["colacc"]).astype(np.float32)
        min2 = -(colacc.max(0).astype(np.float64))
    else:
        # colmins: sorted-p2 index m = 128*k + p  ->  partial[p, 32+k]
        min2 = -partial[:, NSTRIP : 2 * NSTRIP].T.reshape(N).astype(np.float64)

    x1 = s1[:, 0].astype(np.float64)
    x2 = s2[:, 0].astype(np.float64)

    # --- certify rowmins ---
    strip = np.arange(N) // P
    lo_n = lo[strip]
    hi_n = hi[strip]
    bound = np.full(N, np.inf)
    has_left = lo_n > 0
    gl = x1 - np.where(has_left, x2[np.maximum(lo_n - 1, 0)], -np.inf)
    bound = np.where(has_left, np.minimum(bound, np.maximum(gl, 0.0) ** 2), bound)
    has_right = hi_n < N
    gr = np.where(has_right, x2[np.minimum(hi_n, N - 1)], np.inf) - x1
    bound = np.where(
        has_right, np.minimum(bound, np.maximum(gr, 0.0) ** 2), bound
    )
    bad1 = np.nonzero(min1 * (1.0 + 1e-2) + 5e-5 >= bound)[0]
    if bad1.size:
        min1[bad1] = (
            ((s1[bad1, None, :].astype(np.float64) - s2[None, :, :]) ** 2)
            .sum(-1)
            .min(1)
        )

    # --- certify colmins ---
    # column m is covered by strips i with lo_i <= m < hi_i; covered rows
    # are a contiguous range [rlo_m, rhi_m).
    m = np.arange(N)
    # i_lo(m): first strip covering m = first i with hi_i > m
    ilo = np.searchsorted(hi, m, side="right")
    # i_hi(m): last strip covering m = last i with lo_i <= m
    ihi = np.searchsorted(lo, m, side="right") - 1
    rlo = ilo * P
    rhi = (ihi + 1) * P
    bound2 = np.full(N, np.inf)
    hasb = rlo > 0
    gb = x2 - np.where(hasb, x1[np.maximum(rlo - 1, 0)], -np.inf)
    bound2 = np.where(hasb, np.minimum(bound2, np.maximum(gb, 0.0) ** 2), bound2)
    hast = rhi < N
    gt = np.where(hast, x1[np.minimum(rhi, N - 1)], np.inf) - x2
    bound2 = np.where(
        hast, np.minimum(bound2, np.maximum(gt, 0.0) ** 2), bound2
    )
    bad2 = np.nonzero(min2 * (1.0 + 1e-2) + 5e-5 >= bound2)[0]
    if bad2.size:
        min2[bad2] = (
            ((s2[bad2, None, :].astype(np.float64) - s1[None, :, :]) ** 2)
            .sum(-1)
            .min(1)
        )

    return min1.sum(), min2.sum()


def kernel(points1, points2):
    from concourse.bass_utils import run_bass_kernel_spmd

    in_maps, state = _prep(points1, points2)
    nc = get_nc()
    res = run_bass_kernel_spmd(nc, in_maps, list(range(B))).results
    tot = 0.0
    for b in range(B):
        s1, s2 = state[b]
        sum1, sum2 = _postprocess(res[b], s1, s2)
        tot += sum1 + sum2
    loss = tot / (B * N * B)
    return np.float32(loss)


# revision 84
# speedup vs baseline: 17.9506x; 1.5960x over previous
"""Chamfer distance loss kernel for 8 Trainium2 NeuronCores.

Problem: points1 [8, 4096, 3], points2 [8, 4096, 3] (f32).
  dist[b,n,m] = ||p1[b,n]||^2 + ||p2[b,m]||^2 - 2 p1.p2
  loss = (mean_n,b(min_m dist) + mean_m,b(min_n dist)) / 8     (scalar f32)

Sharding: data-parallel over batch B: core b handles batch b.

v2 design (per core):
  Host: sort both clouds by x. Lift each point to K=16 fp16 rows
  (hi/lo split of every coordinate and squared norm, all cross terms)
  so that -d[n,m] = sum_k la[k,n]*lb[k,m] exactly to ~1.5e-5: the
  matmul streams at 1 cyc/col (fp16) instead of fp32's 4 cyc/col.
  Windowing: strip i of 128 sorted points1 only computes distances to
  the W-wide contiguous window of sorted points2 at the same quantile.
  Cuts PE/ACT/DVE work by N/W (W=256 -> 16x fewer elements than the
  dense 4096^2 matrix).
  Device loop over 32 row-strips:
     PE:  matmul (free=W, fp16, K=16) -> PSUM [128, W] f32, rotating
          tile_position bands so LDWEIGHTS overlaps the previous matmul
     ACT: cast PSUM f32 -> SBUF fp16 strip
     DVE: colacc[:, window] = max(colacc, strip)  (colacc pre-set to
          -inf so every update is one uniform TT at fp16 2x mode);
          rowmax fold-max tree W->128 over G=4 strips per op (3D APs)
  Outputs (ship=3): colacc [128, 4096] f16 DMA'd out per quarter as
  soon as its columns are final (overlapped with the loop, 2 rings);
  rowacc folded 128->1 on DVE at the end -> partial [128, 32] f32.
Host post: un-sort; rowmins from partial, colmins = colacc.max(0);
  certify each min against the sorted-x window-edge bound (sound for
  any input); recompute the few uncertified points exactly on host;
  means -> loss.
"""

import sys
import numpy as np

for _p in ("/opt/trn_rl_repo", "/root/.axon_site/_ro/trn_rl_repo"):
    if _p not in sys.path:
        sys.path.insert(0, _p)

B = 8
N = 4096
D = 3
K = 16
P = 128
NSTRIP = N // P          # 32
W = 256                  # window width (columns per strip)

_NC_CACHE = {}


def _window_lo(i, w=None):
    w = w or W
    return min(max(128 * i + 64 - w // 2, 0), N - w)


def _build_nc(
    repeat=1, w=W, parts="full", nmm=None, midtail=0, g=4, paircast=0, ship=3,
    meminit="v", rowx=0, sbufs=3, dvecast=0,
):
    import contextlib

    import concourse.bacc as bacc
    import concourse.tile as tile
    from concourse import mybir

    F16 = mybir.dt.float16
    F32 = mybir.dt.float32
    MAX = mybir.AluOpType.max

    do_act = parts != "mm"
    do_col = parts in ("mm+act+col", "notail", "full")
    do_row = parts in ("mm+act+row", "notail", "full")
    do_tail = parts == "full"

    if nmm is None:
        nmm = max(1, w // 512)  # matmul PSUM writes must be 512-f32 (bank)
    MM_FREE = w // nmm
    G = g
    if do_tail and w > 1024 and not ship:
        # PSUM budget: 2 ph bufs (w*4 B) + 2 tailq bufs (2KB) must fit 16KB
        raise NotImplementedError("full tail only supported for w <= 1024")
    if parts != "full":
        ship = 0  # ablation variants use the plain partial output
    midtail = do_tail and midtail
    # PSUM: 16KB/partition total; tailq ring (2x2KB) reserved when tailing.
    # Matmul PSUM writes must start at a 2KB bank boundary, so each strip
    # gets a full [P, 512] f32 bank even when w < 512.
    pc = 2 if paircast else 1
    phw = max(w, 512)
    phbufs = max(2, min(8, (16384 - (4096 if do_tail else 0)) // (phw * 4 * pc)))
    # last strip whose window intersects colacc quarter q
    qdone = [
        max(
            i
            for i in range(NSTRIP)
            if _window_lo(i, w) < (N // 4) * (q + 1)
        )
        for q in range(4)
    ]
    # last strip whose window intersects 128-col chunk c (for chunked ship)
    cdone = [
        max(i for i in range(NSTRIP) if _window_lo(i, w) < P * (c + 1))
        for c in range(NSTRIP)
    ]
    # column spans with a constant contributor set: between consecutive
    # window breakpoints every column is covered by the same strips, so
    # the column max is a direct k-ary max of strip slices (no RMW chain)
    los = [_window_lo(i, w) for i in range(NSTRIP)]
    breaks = sorted({0, N, *los, *(l + w for l in los)})
    breaks = [b for b in breaks if 0 <= b <= N]
    spans = []  # (a, b, contributors, emit_strip)
    for a, b in zip(breaks[:-1], breaks[1:]):
        contrib = [i for i in range(NSTRIP) if los[i] <= a and b <= los[i] + w]
        assert contrib, f"uncovered span {a}:{b}"
        spans.append((a, b, contrib, max(contrib)))

    nc = bacc.Bacc(
        "TRN2", target_bir_lowering=False, debug=False, num_devices=B
    )
    la = nc.declare_dram_parameter("la", [K, N], F16, isOutput=False)
    lb = nc.declare_dram_parameter("lb", [K, N], F16, isOutput=False)
    ident = nc.declare_dram_parameter("ident", [P, P], F16, isOutput=False)
    if ship in (1, 2, 4):
        out_col = nc.declare_dram_parameter("colacc", [P, N], F16, isOutput=True)
        out_row = nc.declare_dram_parameter("rowacc", [P, N], F16, isOutput=True)
    elif ship in (3, 9):
        out_col = nc.declare_dram_parameter("colacc", [P, N], F16, isOutput=True)
        out = nc.declare_dram_parameter("partial", [P, NSTRIP], F32, isOutput=True)
    else:
        out = nc.declare_dram_parameter(
            "partial", [P, 2 * NSTRIP], F32, isOutput=True
        )

    with tile.TileContext(nc) as tc:
        with (
            tc.tile_pool(name="consts", bufs=1) as consts,
            tc.tile_pool(name="strips", bufs=sbufs) as strips,
            tc.tile_pool(name="scr", bufs=3) as scr,
            tc.tile_pool(name="accs", bufs=1) as accs,
            tc.tile_pool(name="psum", bufs=2, space="PSUM") as psum,
        ):
            la_sb = consts.tile([32 * 3 + K, N], F16)
            lb_sb = consts.tile([32 * 3 + K, N], F16)
            for q in range(4):
                nc.sync.dma_start(out=la_sb[32 * q : 32 * q + K, :], in_=la[:])
                nc.scalar.dma_start(out=lb_sb[32 * q : 32 * q + K, :], in_=lb[:])
            idt = consts.tile([P, P], F16)
            nc.gpsimd.dma_start(out=idt[:], in_=ident[:])

            # double-buffer per-iteration state in the repeat loop so the
            # ship DMAs / re-init of iteration k never serialize against
            # iteration k+1's accumulator writes (steady-state = single-shot)
            nbuf = 2 if (ship in (3, 4) and do_tail and repeat > 1) else 1
            if nbuf == 2:
                assert repeat % 2 == 0, "repeat must be even for dbuf"
            colaccs, rowaccs, rowacc32s, summs = [], [], [], []
            for u in range(nbuf):
                colaccs.append(
                    accs.tile([P, N], F16, name=f"colacc{u}", tag=f"colacc{u}")
                )
                rowaccs.append(
                    accs.tile(
                        [P, NSTRIP * P], F16, name=f"rowacc{u}", tag=f"rowacc{u}"
                    )
                )
                rowacc32s.append(
                    accs.tile(
                        [P, NSTRIP * 32],
                        F16,
                        name=f"rowacc32{u}",
                        tag=f"rowacc32{u}",
                    )
                )
                summs.append(
                    accs.tile(
                        [P, 2 * NSTRIP], F32, name=f"summ{u}", tag=f"summ{u}"
                    )
                )
                if do_col and ship not in (3, 4):
                    # pre-init to -inf so every window update is one uniform
                    # full-width max (gpsimd is otherwise idle)
                    for q in range(4):
                        nc.gpsimd.memset(
                            colaccs[u][:, q * (N // 4) : (q + 1) * (N // 4)],
                            -60000.0,
                        )
            if ship == 9:  # hazard diagnostic: ship from a constant tile
                dummy = accs.tile([P, N], F16)
                nc.vector.memset(dummy[:], 0.0)

            def emit_iteration(colacc, rowacc, rowacc32, summ):
                def emit_quarter(q):
                    tailq = psum.tile([P, 8, P], F16, tag="tailq")
                    for t in range(8):
                        k = 8 * q + t
                        nc.tensor.transpose(
                            tailq[:, t, :],
                            colacc[:, k * P : (k + 1) * P],
                            idt[:],
                        )
                    nc.vector.tensor_reduce(
                        out=summ[:, NSTRIP + 8 * q : NSTRIP + 8 * q + 8],
                        in_=tailq[:],
                        axis=mybir.AxisListType.X,
                        op=MAX,
                    )
                    # re-init for the next repeat iteration
                    me = nc.vector if meminit == "v" else nc.gpsimd
                    me.memset(
                        colacc[:, q * (N // 4) : (q + 1) * (N // 4)], -60000.0
                    )

                strip_ap = {}
                for ip in range(NSTRIP // G):
                    dstrip = strips.tile([P, G, w], F16, tag="strip")
                    for _s in range(G):
                        strip_ap[G * ip + _s] = (dstrip, _s)
                    ph_pair = None
                    for s in range(G):
                        i = G * ip + s
                        lo = _window_lo(i, w)
                        if pc == 1:
                            ph_full = psum.tile(
                                [P, phw], F32, tag="ph", bufs=phbufs
                            )
                            ph = ph_full[:, 0:w]
                        elif s % 2 == 0:
                            ph_pair = psum.tile(
                                [P, 2, phw], F32, tag="ph", bufs=phbufs
                            )
                            ph = ph_pair[:, 0, 0:w]
                        else:
                            ph = ph_pair[:, 1, 0:w]
                        for j in range(nmm):
                            # rotate tile-position bands across strips so
                            # LDWEIGHTS(i+1) overlaps MATMUL(i) (different
                            # row groups -> PE pulls the load ahead)
                            band = (i * nmm + j) % 4
                            m0 = j * MM_FREE
                            nc.tensor.matmul(
                                ph[:, m0 : m0 + MM_FREE],
                                lhsT=la_sb[
                                    32 * band : 32 * band + K,
                                    i * P : (i + 1) * P,
                                ],
                                rhs=lb_sb[
                                    32 * band : 32 * band + K,
                                    lo + m0 : lo + m0 + MM_FREE,
                                ],
                                start=True,
                                stop=True,
                                tile_position=(32 * band, 0),
                            )
                        def colship(ii, ss):
                            llo = _window_lo(ii, w)
                            if do_col and ship in (3, 4):
                                # write-only colacc: each span's column max is
                                # a direct k-ary max over its contributor
                                # strips' slices (no RMW chain, no memsets)
                                for sidx, (a, b, contrib, emit) in enumerate(
                                    spans
                                ):
                                    if emit != ii:
                                        continue
                                    srcs = []
                                    for ci in contrib:
                                        t, tsl = strip_ap[ci]
                                        off = a - los[ci]
                                        srcs.append(
                                            t[:, tsl, off : off + (b - a)]
                                        )
                                    if len(srcs) == 1:
                                        nc.vector.tensor_copy(
                                            colacc[:, a:b], srcs[0]
                                        )
                                    else:
                                        nc.vector.tensor_tensor(
                                            colacc[:, a:b],
                                            srcs[0],
                                            srcs[1],
                                            op=MAX,
                                        )
                                        for extra in srcs[2:]:
                                            nc.vector.tensor_tensor(
                                                colacc[:, a:b],
                                                colacc[:, a:b],
                                                extra,
                                                op=MAX,
                                            )
                                    eng = (
                                        nc.sync if sidx % 2 == 0 else nc.gpsimd
                                    )
                                    eng.dma_start(
                                        out=out_col[:, a:b],
                                        in_=colacc[:, a:b],
                                    )
                                return
                            if do_col:
                                # colacc pre-set to -inf: uniform window max
                                nc.vector.tensor_tensor(
                                    colacc[:, llo : llo + w],
                                    colacc[:, llo : llo + w],
                                    dstrip[:, ss, :],
                                    op=MAX,
                                )
                            if do_tail and ship in (1, 9):
                                # ship each 128-col chunk the moment its
                                # columns are final (no end-exposed DMA)
                                for c in range(NSTRIP):
                                    if cdone[c] == ii:
                                        cs = slice(c * P, (c + 1) * P)
                                        eng = (
                                            nc.sync
                                            if c % 2 == 0
                                            else nc.gpsimd
                                        )
                                        src = dummy if ship == 9 else colacc
                                        eng.dma_start(
                                            out=out_col[:, cs], in_=src[:, cs]
                                        )
                                for q in range(4):
                                    if qdone[q] == ii:
                                        qs = slice(
                                            q * (N // 4), (q + 1) * (N // 4)
                                        )
                                        # re-init for this buffer's next use
                                        me = (
                                            nc.vector
                                            if meminit == "v"
                                            else nc.gpsimd
                                        )
                                        me.memset(colacc[:, qs], -60000.0)
                            elif midtail:
                                for q in range(4):
                                    if qdone[q] == ii:
                                        emit_quarter(q)

                        # spread a few casts onto DVE's slack to
                        # rebalance the ACT-paced pipeline
                        on_dve = dvecast and i % (NSTRIP // dvecast) == 2
                        if pc == 1:
                            if do_act and on_dve:
                                nc.vector.tensor_copy(dstrip[:, s, :], ph[:])
                            elif do_act:
                                nc.scalar.copy(dstrip[:, s, :], ph[:])
                            colship(i, s)
                        elif s % 2 == 1:
                            if do_act:
                                nc.scalar.copy(
                                    dstrip[:, s - 1 : s + 1, :],
                                    ph_pair[:, :, 0:w],
                                )
                            colship(i - 1, s - 1)
                            colship(i, s)
                    if do_row:
                        v = w
                        fsrc = dstrip[:]
                        while v > 2 * P:
                            ww = v // 2
                            fdst = scr.tile([P, G, ww], F16, tag=f"fold{ww}")
                            nc.vector.tensor_tensor(
                                fdst[:],
                                fsrc[:, :, 0:ww],
                                fsrc[:, :, ww:v],
                                op=MAX,
                            )
                            fsrc = fdst[:]
                            v = ww
                        i0 = G * ip
                        # overlapped final halves (max is idempotent) so any
                        # 128 < v <= 256 reduces to the 128-wide rowacc slot
                        nc.vector.tensor_tensor(
                            rowacc[:, i0 * P : (i0 + G) * P].rearrange(
                                "p (s q) -> p s q", s=G
                            ),
                            fsrc[:, :, 0:P],
                            fsrc[:, :, v - P : v],
                            op=MAX,
                        )
                        if rowx:
                            # fold this group's rowmax candidates 128 -> 32
                            # in-loop (DVE slack) to shrink the end rfold
                            rsrc = rowacc[
                                :, i0 * P : (i0 + G) * P
                            ].rearrange("p (s q) -> p s q", s=G)
                            r64 = scr.tile([P, G, 64], F16, tag="rx64")
                            nc.vector.tensor_tensor(
                                r64[:], rsrc[:, :, 0:64], rsrc[:, :, 64:128],
                                op=MAX,
                            )
                            nc.vector.tensor_tensor(
                                rowacc32[:, i0 * 32 : (i0 + G) * 32].rearrange(
                                    "p (s q) -> p s q", s=G
                                ),
                                r64[:, :, 0:32],
                                r64[:, :, 32:64],
                                op=MAX,
                            )
                        if do_tail and ship == 1:
                            nc.gpsimd.dma_start(
                                out=out_row[:, i0 * P : (i0 + G) * P],
                                in_=rowacc[:, i0 * P : (i0 + G) * P],
                            )
                        elif do_tail and ship == 4:
                            # raw rowmax candidates out on the ACT-issued
                            # ring; host does the 128-way fold
                            nc.scalar.dma_start(
                                out=out_row[:, i0 * P : (i0 + G) * P],
                                in_=rowacc[:, i0 * P : (i0 + G) * P],
                            )

                # ---- tail ----
                if do_tail and ship == 4:
                    pass  # everything shipped incrementally
                elif do_tail and ship in (3, 9):
                    if rowx:
                        rw = 16
                        v = rowacc32[:].rearrange("p (i q) -> p i q", q=32)
                    else:
                        rw = P // 2
                        v = rowacc[:].rearrange("p (i q) -> p i q", q=P)
                    while rw >= 2:
                        rdst = scr.tile([P, NSTRIP, rw], F16, tag=f"rfold{rw}")
                        nc.vector.tensor_tensor(
                            rdst[:], v[:, :, 0:rw], v[:, :, rw : 2 * rw], op=MAX
                        )
                        v = rdst[:]
                        rw //= 2
                    nc.vector.tensor_reduce(
                        out=summ[:, 0:NSTRIP],
                        in_=v,
                        axis=mybir.AxisListType.X,
                        op=MAX,
                    )
                    nc.sync.dma_start(out=out[:], in_=summ[:, 0:NSTRIP])
                elif do_tail and ship == 2:
                    for q in range(4):
                        nc.sync.dma_start(
                            out=out_col[:, q * (N // 4) : (q + 1) * (N // 4)],
                            in_=colacc[:, q * (N // 4) : (q + 1) * (N // 4)],
                        )
                        nc.gpsimd.dma_start(
                            out=out_row[:, q * (N // 4) : (q + 1) * (N // 4)],
                            in_=rowacc[:, q * (N // 4) : (q + 1) * (N // 4)],
                        )
                elif do_tail:
                    if not midtail:
                        for q in range(4):
                            emit_quarter(q)
                    # per-strip rowmax: fold the 128 candidates to 2, reduce
                    rw = P // 2
                    v = rowacc[:].rearrange("p (i q) -> p i q", q=P)
                    while rw >= 2:
                        rdst = scr.tile([P, NSTRIP, rw], F16, tag=f"rfold{rw}")
                        nc.vector.tensor_tensor(
                            rdst[:], v[:, :, 0:rw], v[:, :, rw : 2 * rw], op=MAX
                        )
                        v = rdst[:]
                        rw //= 2
                    nc.vector.tensor_reduce(
                        out=summ[:, 0:NSTRIP],
                        in_=v,
                        axis=mybir.AxisListType.X,
                        op=MAX,
                    )
                    nc.sync.dma_start(out=out[:], in_=summ[:])
                elif not ship:
                    zer = accs.tile([P, 2 * NSTRIP], F32, tag="zer")
                    nc.vector.memset(zer[:], 0.0)
                    nc.sync.dma_start(out=out[:], in_=zer[:])

            if repeat == 1:
                emit_iteration(colaccs[0], rowaccs[0], rowacc32s[0], summs[0])
            else:
                with tc.For_i(0, repeat // nbuf, 1):
                    for u in range(nbuf):
                        emit_iteration(
                            colaccs[u], rowaccs[u], rowacc32s[u], summs[u]
                        )

    nc.compile()
    return nc


def get_nc(
    repeat=1,
    w=W,
    parts="full",
    nmm=None,
    midtail=0,
    g=4,
    paircast=0,
    ship=3,
    meminit="v",
    rowx=0,
    sbufs=3,
    dvecast=0,
    **_ignored,
):
    key = (
        repeat, w, parts, nmm, midtail, g, paircast, ship, meminit, rowx,
        sbufs, dvecast,
    )
    if key not in _NC_CACHE:
        _NC_CACHE[key] = _build_nc(
            repeat=repeat,
            w=w,
            parts=parts,
            nmm=nmm,
            midtail=midtail,
            g=g,
            paircast=paircast,
            ship=ship,
            meminit=meminit,
            rowx=rowx,
            sbufs=sbufs,
            dvecast=dvecast,
        )
    return _NC_CACHE[key]


def _f16(x):
    return x.astype(np.float16).astype(np.float32)


def _lift(p1, p2):
    """fp16 hi/lo lifted vectors (sorted clouds) so -dist = la^T @ lb.

    All 16 rows are exactly representable in fp16; the matmul in fp16
    with fp32 accumulate reproduces -d to ~1.5e-5 abs.
    """
    sq1 = (p1 * p1).sum(-1)
    sq2 = (p2 * p2).sum(-1)
    la = np.zeros((K, N), np.float32)
    lb = np.zeros((K, N), np.float32)
    s1h = _f16(sq1)
    s2h = _f16(sq2)
    la[0] = s1h
    lb[0] = -1.0
    la[1] = _f16(sq1 - s1h)
    lb[1] = -1.0
    la[2] = 1.0
    lb[2] = -s2h
    la[3] = 1.0
    lb[3] = -_f16(sq2 - s2h)
    for d in range(D):
        x = p1[:, d]
        y = p2[:, d]
        xh = _f16(x)
        xl = _f16(x - xh)
        yh = _f16(2.0 * y)
        yl = _f16(2.0 * y - yh)
        base = 4 + 4 * d
        la[base + 0] = xh
        lb[base + 0] = yh
        la[base + 1] = xh
        lb[base + 1] = yl
        la[base + 2] = xl
        lb[base + 2] = yh
        la[base + 3] = xl
        lb[base + 3] = yl
    return la.astype(np.float16), lb.astype(np.float16)


def _prep(points1, points2):
    """Sort by x per batch; return per-core input maps + sort state."""
    p1 = np.asarray(points1, dtype=np.float32)
    p2 = np.asarray(points2, dtype=np.float32)
    ident = np.eye(P, dtype=np.float16)
    in_maps = []
    state = []
    for b in range(B):
        o1 = np.argsort(p1[b, :, 0], kind="stable")
        o2 = np.argsort(p2[b, :, 0], kind="stable")
        s1 = p1[b][o1]
        s2 = p2[b][o2]
        la, lb = _lift(s1, s2)
        in_maps.append(
            {
                "la": np.ascontiguousarray(la),
                "lb": np.ascontiguousarray(lb),
                "ident": ident,
            }
        )
        state.append((s1, s2))
    return in_maps, state


def _in_maps(points1, points2):
    return _prep(points1, points2)[0]


def _postprocess(res, s1, s2, w=W):
    """Un-sort device maxes, certify vs window-edge bounds, fix up.

    res: one core's output dict (negated maxes).
    Returns (sum_min1, sum_min2) exact sums of per-point NN^2.
    """
    lo = np.array([_window_lo(i, w) for i in range(NSTRIP)])
    hi = lo + w
    if "partial" in res:
        partial = np.asarray(res["partial"])
        # rowmins: sorted-p1 index n = 128*i + p  ->  partial[p, i]
        min1 = -partial[:, 0:NSTRIP].T.reshape(N).astype(np.float64)
    else:
        # rowacc[p, i*128 + j]: candidate j for sorted-p1 index 128*i + p
        rowacc = np.asarray(res["rowacc"]).astype(np.float32)
        min1 = -(
            rowacc.reshape(P, NSTRIP, P).max(-1).T.reshape(N).astype(np.float64)
        )
    if "colacc" in res:
        # colacc[p, m]: strip-lane p candidate for sorted-p2 index m
        colacc = np.asarray(res["colacc"]).astype(np.float32)
        min2 = -(colacc.max(0).astype(np.float64))
    else:
        # colmins: sorted-p2 index m = 128*k + p  ->  partial[p, 32+k]
        min2 = -partial[:, NSTRIP : 2 * NSTRIP].T.reshape(N).astype(np.float64)

    x1 = s1[:, 0].astype(np.float64)
    x2 = s2[:, 0].astype(np.float64)

    # --- certify rowmins ---
    strip = np.arange(N) // P
    lo_n = lo[strip]
    hi_n = hi[strip]
    bound = np.full(N, np.inf)
    has_left = lo_n > 0
    gl = x1 - np.where(has_left, x2[np.maximum(lo_n - 1, 0)], -np.inf)
    bound = np.where(has_left, np.minimum(bound, np.maximum(gl, 0.0) ** 2), bound)
    has_right = hi_n < N
    gr = np.where(has_right, x2[np.minimum(hi_n, N - 1)], np.inf) - x1
    bound = np.where(
        has_right, np.minimum(bound, np.maximum(gr, 0.0) ** 2), bound
    )
    bad1 = np.nonzero(min1 * (1.0 + 1e-2) + 5e-5 >= bound)[0]
    if bad1.size:
        min1[bad1] = (
            ((s1[bad1, None, :].astype(np.float64) - s2[None, :, :]) ** 2)
            .sum(-1)
            .min(1)
        )

    # --- certify colmins ---
    # column m is covered by strips i with lo_i <= m < hi_i; covered rows
    # are a contiguous range [rlo_m, rhi_m).
    m = np.arange(N)
    # i_lo(m): first strip covering m = first i with hi_i > m
    ilo = np.searchsorted(hi, m, side="right")
    # i_hi(m): last strip covering m = last i with lo_i <= m
    ihi = np.searchsorted(lo, m, side="right") - 1
    rlo = ilo * P
    rhi = (ihi + 1) * P
    bound2 = np.full(N, np.inf)
    hasb = rlo > 0
    gb = x2 - np.where(hasb, x1[np.maximum(rlo - 1, 0)], -np.inf)
    bound2 = np.where(hasb, np.minimum(bound2, np.maximum(gb, 0.0) ** 2), bound2)
    hast = rhi < N
    gt = np.where(hast, x1[np.minimum(rhi, N - 1)], np.inf) - x2
    bound2 = np.where(
        hast, np.minimum(bound2, np.maximum(gt, 0.0) ** 2), bound2
    )
    bad2 = np.nonzero(min2 * (1.0 + 1e-2) + 5e-5 >= bound2)[0]
    if bad2.size:
        min2[bad2] = (
            ((s2[bad2, None, :].astype(np.float64) - s1[None, :, :]) ** 2)
            .sum(-1)
            .min(1)
        )

    return min1.sum(), min2.sum()


def kernel(points1, points2):
    from concourse.bass_utils import run_bass_kernel_spmd

    in_maps, state = _prep(points1, points2)
    nc = get_nc()
    res = run_bass_kernel_spmd(nc, in_maps, list(range(B))).results
    tot = 0.0
    for b in range(B):
        s1, s2 = state[b]
        sum1, sum2 = _postprocess(res[b], s1, s2)
        tot += sum1 + sum2
    loss = tot / (B * N * B)
    return np.float32(loss)
